# revision 46
# baseline (speedup 1.0000x reference)
"""Self-contained Trainium2 Bass kernel for Canny edge detection (4,3,1024,1024).

kernel(x) -> (magnitude, edges), each [4,1,1024,1024] f32. 8 NeuronCores SPMD:
core = (batch, image half); no cross-core communication (flood-fill halo margin).
"""
import numpy as np

XR = 532          # x window rows per core
NM = 529          # mag rows per core
H_IMG, W_IMG = 1024, 1024
RD = 532          # r-dim of mag-grid col-major buffers: slot = 1+M, guards at 0,530
WSLOT = 19        # flood word slots: 0 guard, 1..17 data, 18 guard
PACK_ROWS = 544   # 17 words * 32 rows
ITERS = 4
GRAY_W = np.array([0.299, 0.587, 0.114], np.float32)
TAN225 = np.float32(np.tan(np.pi / 8))  # 0.41421356

def thresh2(c):
    """Largest f32 v* with (v > v*) == (f32(sqrt(v)) > c) for f32 v; NMS thresholds on m2+eps."""
    c = np.float32(c)
    v = np.float32(c * c)
    while np.float32(np.sqrt(v)) > c:
        v = np.nextafter(v, np.float32(0), dtype=np.float32)
    while np.float32(np.sqrt(np.nextafter(v, np.float32(np.inf), dtype=np.float32))) <= c:
        v = np.nextafter(v, np.float32(np.inf), dtype=np.float32)
    return float(v)

def gauss5():
    # f32 replica of reference._gaussian_kernel1d(5, 1.0)
    x = (np.arange(5, dtype=np.float32) - 2).astype(np.float32)
    g = np.exp((-x * x / np.float32(2.0)).astype(np.float32)).astype(np.float32)
    return (g / g.sum(dtype=np.float32)).astype(np.float32)

def _op_conv(n_out, n_in, taps, center, idx_map):
    """Row t of output = sum_d taps[d] * in[idx_map(t + d - center)], f64 build."""
    C = np.zeros((n_out, n_in), np.float64)
    for t in range(n_out):
        for d, w in enumerate(taps):
            s = idx_map(t + d - center)
            C[t, s] += w
    return C

def reflect_idx(i, n):
    # jnp.pad 'reflect': -1 -> 1, -2 -> 2; n -> n-2, n+1 -> n-3
    if i < 0:
        return -i
    if i >= n:
        return 2 * n - 2 - i
    return i

def clamp_idx(i, n):
    return min(max(i, 0), n - 1)

def build_vertical_ops(top: bool):
    """Return (Cvx, Cvy): [NM, XR] f32 composed vertical operators for this core."""
    g = gauss5().astype(np.float64)
    # Stage 1: gauss vertical with reflect at IMAGE edges, over x window rows.
    # blurred-v needed rows: image rows of M-1 .. M+1 -> local B = -1..529
    # local->img: top: img = local_x;  bottom: img = 492 + local_x
    # blurV local grid b = -1..529 maps to img rows (top: b, bottom: 492+b... wait
    #   bottom mag M -> img 495+M; blur rows needed img 494..1024)
    # Build on local-x axis directly with the correct edge behavior:
    #   top: local 0 == img 0 (reflect boundary at local 0); far end interior.
    #   bottom: local 531 == img 1023 (reflect boundary there); near end interior.
    NB = 531  # blur rows b = -1..529 stored t = b+1
    def xmap_top(i):   # reflect at 0 only (other end never reached out of range)
        return reflect_idx(i, 10**9) if i >= 0 else -i
    def xmap_bot(i):
        if i >= XR:
            return 2 * XR - 2 - i
        return i
    xmap = xmap_top if top else xmap_bot
    # blur b (local-x coordinate of output): top: b = t-1; bottom: b = t-1+2
    #   top: blurV[b] centered at x local row b;  b from -1..529
    #   bottom: mag M -> img 495+M -> local x = 495+M-492 = 3+M; blur rows local x = 2+M-1.. 
    #     blur grid b(local x) = 2 .. 532 for t=0..530
    off = 0 if top else 3
    Cb = np.zeros((NB, XR), np.float64)
    for t in range(NB):
        b = t - 1 + off   # local-x row this blurV output is centered on
        for d in range(5):
            s = b + d - 2
            s = xmap(s)
            assert 0 <= s < XR, (top, t, s)
            Cb[t, s] += g[d]
    # Stage 2: sobel vertical ops on the blurV grid with replicate at IMAGE edges.
    # mag M: taps at blur rows b = (M-1 .. M+1) in local-x => stored t = M-1+1..=M..M+2 - wait
    #   stored t of blur row b: t = b + 1 - off ... b_local_x = M + off + db where db=-1..1
    #   stored t = (M + off + db) - off + 1 - ... let me just: stored t corresponds to b_lx = t-1+off
    #   For mag M: need b_lx = (M+off) + db  => t = M + 1 + db
    # replicate at image edges: top: b_lx < 0 -> 0 i.e. t<0 -> t=0? replicate on blur IMG rows:
    #   top: blur img row = b_lx; replicate row<0 -> row 0 -> t index of b_lx=0 is t=1.
    #   bottom: blur img row = 492 + b_lx; replicate row>1023 -> b_lx>531 -> clamp to 531 (t=530)
    vsm = np.array([1.0, 2.0, 1.0])
    vdf = np.array([-1.0, 0.0, 1.0])
    Cvx = np.zeros((NM, XR), np.float64)
    Cvy = np.zeros((NM, XR), np.float64)
    for M in range(NM):
        for db, (wx, wy) in enumerate(zip(vsm, vdf)):
            t = M + db  # t = M+1+(db-1)
            if top:
                t = max(t, 1)       # replicate img row 0 (t=1)... t=0 is b_lx=-1 (img -1)
            else:
                t = min(t, NB - 2)  # replicate img row 1023 at far end (t=529)
            # also clamp other end (never used beyond range by construction)
            t = min(max(t, 0), NB - 1)
            Cvx[M] += wx * Cb[t]
            Cvy[M] += wy * Cb[t]
    return Cvx.astype(np.float32), Cvy.astype(np.float32)

def build_horizontal_ops():
    """(Chx, Chy): [W, W] composed horizontal operators (same both cores)."""
    g = gauss5().astype(np.float64)
    Cb = _op_conv(W_IMG, W_IMG, g, 2, lambda i: reflect_idx(i, W_IMG))
    Dif = _op_conv(W_IMG, W_IMG, [-1.0, 0.0, 1.0], 1, lambda i: clamp_idx(i, W_IMG))
    Sm = _op_conv(W_IMG, W_IMG, [1.0, 2.0, 1.0], 1, lambda i: clamp_idx(i, W_IMG))
    Chx = (Dif @ Cb).astype(np.float32)
    Chy = (Sm @ Cb).astype(np.float32)
    return Chx, Chy

# ---------------- numpy model of the per-core pipeline (for validation) -------------
def core_model(x_win, top):
    """x_win: [3, XR, 1024] f32. Returns (magout [NM,1024], edges [NM,1024])."""
    Cvx, Cvy = build_vertical_ops(top)
    Chx, Chy = build_horizontal_ops()
    gray = np.tensordot(GRAY_W, x_win.astype(np.float32), 1)  # [XR, W]
    gvx = (Cvx @ gray).astype(np.float32)
    gvy = (Cvy @ gray).astype(np.float32)
    gx = (gvx @ Chx.T).astype(np.float32)
    gy = (gvy @ Chy.T).astype(np.float32)
    m2 = gx * gx + gy * gy
    mag = np.sqrt(m2 + np.float32(1e-6)).astype(np.float32)
    magp = np.zeros((NM + 2, W_IMG + 2), np.float32)
    magp[1:-1, 1:-1] = mag
    ax, ay = np.abs(gx), np.abs(gy)
    maskH = (TAN225 * ax) >= ay
    maskV = (TAN225 * ay) > ax
    pmask = (gx * gy) >= 0
    c = magp[1:-1, 1:-1]
    up, dn = magp[0:-2, 1:-1], magp[2:, 1:-1]
    lf, rt = magp[1:-1, 0:-2], magp[1:-1, 2:]
    ul, ur = magp[0:-2, 0:-2], magp[0:-2, 2:]
    dl, dr = magp[2:, 0:-2], magp[2:, 2:]
    nbH = np.maximum(lf, rt); nbV = np.maximum(up, dn)
    nbD1 = np.maximum(dr, ul); nbD2 = np.maximum(dl, ur)
    nbsel = nbD2.copy()
    nbsel[pmask] = nbD1[pmask]
    nbsel[maskV] = nbV[maskV]
    nbsel[maskH] = nbH[maskH]
    ismax = c > nbsel
    magout = mag * ismax
    sm = magout > np.float32(0.2)
    wm = magout > np.float32(0.1)
    S = sm.copy(); W = wm
    for _ in range(ITERS):
        Sp = np.zeros((NM + 2, W_IMG + 2), bool)
        Sp[1:-1, 1:-1] = S
        dil = Sp[0:-2,0:-2]|Sp[0:-2,1:-1]|Sp[0:-2,2:]|Sp[1:-1,0:-2]|Sp[1:-1,1:-1]|Sp[1:-1,2:]|Sp[2:,0:-2]|Sp[2:,1:-1]|Sp[2:,2:]
        S = S | (W & dil)
    return magout, S.astype(np.float32)


import numpy as np
from collections import defaultdict
import concourse.bass as bass
import concourse.mybir as mybir
from concourse.masks import make_identity


F32, I32, U32, U8 = mybir.dt.float32, mybir.dt.int32, mybir.dt.uint32, mybir.dt.uint8
F32R = mybir.dt.float32r
BF16 = mybir.dt.bfloat16
OP = mybir.AluOpType
ACT = mybir.ActivationFunctionType
MASK_DT = U8

NT = 5
BTS = [128, 128, 128, 128, 17]
NCB = 8
RCH = [(1, 162), (162, 354), (354, 530)]     # conv r-slot chunks
NCH4 = [(1, 161), (161, 353), (353, 530)]    # NMS r-slot chunks (word-aligned)
PKW = [(0, 5), (5, 11), (11, 17)]            # pack word ranges per NMS chunk

XP = 535           # padded x rows: top = [0,0,0, img 0..531]; bottom = [img 492..1023, 0,0,0]
VBKS = [122, 122, 122, 122, 41]   # output rows per vertical block (sum 529)
VKS = [128, 128, 128, 128, 47]    # input rows per block, start = 122*k

def build_vplan(top):
    Cvx, Cvy = build_vertical_ops(top)
    w = np.float64(np.float32(GRAY_W[2]))  # 0.114 folded out of the DVE gray stage
    # pad to the unified 535-row local axis
    pads = []
    for C in (Cvx, Cvy):
        Cp = np.zeros((NM, XP), np.float64)
        if top:
            Cp[:, 3:3 + XR] = C
        else:
            Cp[:, 0:XR] = C
        pads.append(Cp)
    arr = np.zeros((128, 2 * NT, 128), np.float32)
    for ci, C in enumerate(pads):
        for k in range(NT):
            r0, BK, K = 122 * k, VBKS[k], VKS[k]
            sub = C[r0:r0 + BK, :]
            assert np.all(sub[:, :122 * k] == 0) and np.all(sub[:, 122 * k + K:] == 0), (top, ci, k)
            arr[0:K, ci * NT + k, 0:BK] = (w * sub[:, 122 * k:122 * k + K]).T
    return arr

def build_hplan():
    """wh [128, 16, 128]: slot ci*8+cb = dense diagonal block (input cols 128cb..+127).
    whx [8, 16, 128]: halo block: rows 0..2 = input cols 128cb-3..-1, rows 3..5 = 128cb+128..+130."""
    Chx, Chy = build_horizontal_ops()
    wh = np.zeros((128, 2 * NCB, 128), np.float32)
    whx = np.zeros((64, NCB, 128), np.float32)
    for ci, C in enumerate((Chx, Chy)):
        for cb in range(NCB):
            p0 = 128 * cb
            s = ci * NCB + cb
            b = 32 * ci
            wh[:, s, :] = C[p0:p0 + 128, p0:p0 + 128].T
            if cb > 0:
                whx[b:b + 3, cb, :] = C[p0:p0 + 128, p0 - 3:p0].T
            if cb < NCB - 1:
                whx[b + 3:b + 6, cb, :] = C[p0:p0 + 128, p0 + 128:p0 + 131].T
            assert np.all(C[p0:p0 + 128, :max(p0 - 3, 0)] == 0)
            assert np.all(C[p0:p0 + 128, p0 + 131:] == 0)
    return wh, whx

def pack_blocks(blocks, kinds):
    """kinds[i] in {'full','lo32','hi32'}; hi32 must land at k0=96, lo32/full at 0."""
    places = [None] * len(blocks)
    slots = []
    free_lo, free_hi = [], []
    for i, (b, kind) in enumerate(zip(blocks, kinds)):
        if kind == 'full':
            slots.append([])
            slots[-1].append((0, b))
            places[i] = (len(slots) - 1, 0)
        elif kind == 'lo32':
            if not free_lo:
                slots.append([])
                free_hi.append(len(slots) - 1)
                free_lo.append(len(slots) - 1)
            s = free_lo.pop(0)
            slots[s].append((0, b))
            places[i] = (s, 0)
        else:  # hi64 at k0=64
            if not free_hi:
                slots.append([])
                free_lo.append(len(slots) - 1)
                free_hi.append(len(slots) - 1)
            s = free_hi.pop(0)
            slots[s].append((64, b))
            places[i] = (s, 64)
    arr = np.zeros((128, len(slots), 128), np.float32)
    for slot, entries in enumerate(slots):
        for k0, b in entries:
            K, M = b.shape
            arr[k0:k0 + K, slot, 0:M] = b
    return arr, places

def make_core_inputs(top):
    wv = build_vplan(top)
    wh, whx = build_hplan()
    pat = np.tile(np.uint32(1) << np.arange(32, dtype=np.uint32), (128, 1))
    meta = dict(nv=wv.shape[1], nh=wh.shape[1])
    return np.ascontiguousarray(pat), wv, wh, whx, meta

def _ap(base_ap, offset_elems, dims):
    return bass.AP(base_ap.tensor, base_ap.offset + offset_elems, dims)

def r32(ap):
    return ap.bitcast(mybir.dt.float32r)

def stt_u32(nc, out, in0, scalar, in1, op0, op1):
    """scalar_tensor_tensor with an integer (u32) immediate, for bitvec ops."""
    eng = nc.vector
    return eng.add_instruction(
        mybir.InstTensorScalarPtr(
            name=nc.get_next_instruction_name(),
            is_scalar_tensor_tensor=True,
            op0=op0,
            op1=op1,
            ins=[eng.lower_ap(in0),
                 mybir.ImmediateValue(dtype=mybir.dt.uint32, value=scalar),
                 eng.lower_ap(in1)],
            outs=[eng.lower_ap(out)],
        ))

def canny_core(ctx, tc, outs, ins, meta):
    import os
    STAGE = int(os.environ.get('CANNY_STAGE', '9'))
    from contextlib import ExitStack
    nc = tc.nc
    mag_out, edges_out = outs
    x_in, wv_in, wh_in, whx_in, pat_in, ident_in = ins
    NVS, NHS = meta['nv'], meta['nh']

    consts = ctx.enter_context(tc.tile_pool(name="consts", bufs=1))
    pat_s = consts.tile([128, 32], U32)
    nc.sync.dma_start(pat_s[:], pat_in)
    ident = consts.tile([128, 128], F32R)
    nc.sync.dma_start(ident[:], ident_in)
    identb = consts.tile([128, 128], BF16)
    make_identity(nc, identb)

    persist = ctx.enter_context(tc.tile_pool(name="persist", bufs=1))
    magb = persist.tile([128, NCB, RD], F32)          # holds m2 = gx^2+gy^2
    magob = persist.tile([128, NCB, RD], BF16)        # final masked magnitude (bf16 ok: post-decision values)
    nc.gpsimd.memset(magb[:, :, 0:1], 0.0)
    nc.gpsimd.memset(magb[:, :, 530:532], 0.0)
    m2L_s = persist.tile([128, NCB, 194], F32, name="m2L")
    m2R_s = persist.tile([128, NCB, 194], F32, name="m2R")
    m2L = [m2L_s, m2L_s, m2L_s]
    m2R = [m2R_s, m2R_s, m2R_s]
    nc.gpsimd.memset(m2L_s[0:1, 0:1, :], 0.0)
    nc.gpsimd.memset(m2R_s[96:128, 7:8, :], 0.0)

    swp = ctx.enter_context(tc.tile_pool(name="swp", bufs=1))
    smb = swp.tile([128, NCB, PACK_ROWS + 2], U8)
    wmb = swp.tile([128, NCB, PACK_ROWS + 2], U8)
    nc.gpsimd.memset(smb[:, :, 530:546], 0)
    nc.gpsimd.memset(wmb[:, :, 530:546], 0)
    fl = ctx.enter_context(tc.tile_pool(name="fl", bufs=1))
    Sw = fl.tile([128, NCB, WSLOT], U32, tag="Sw")
    Ww = fl.tile([128, NCB, WSLOT], U32, tag="Ww")
    HL = fl.tile([128, NCB, WSLOT], U32, tag="HL")
    HR = fl.tile([128, NCB, WSLOT], U32, tag="HR")
    Hd = fl.tile([128, NCB, WSLOT], U32, tag="Hd")
    Vd = fl.tile([128, NCB, WSLOT], U32, tag="Vd")
    ta = fl.tile([128, NCB, WSLOT], U32, tag="ta")
    for t in (Sw, Ww, HL, HR, Hd, Vd, ta):
        nc.gpsimd.memset(t[:], 0)
    def flood_iter(dw):
        a, b = dw.start, dw.stop
        dm, dp = slice(a - 1, b - 1), slice(a + 1, b + 1)
        nc.sync.dma_start(out=HL[1:128, :, dw], in_=Sw[0:127, :, dw])
        nc.scalar.dma_start(out=HL[0:1, 1:8, dw], in_=Sw[127:128, 0:7, dw])
        nc.gpsimd.dma_start(out=HR[0:127, :, dw], in_=Sw[1:128, :, dw])
        nc.scalar.dma_start(out=HR[127:128, 0:7, dw], in_=Sw[0:1, 1:8, dw])
        nc.vector.tensor_tensor(out=Hd[:, :, dw], in0=Sw[:, :, dw], in1=HL[:, :, dw], op=OP.bitwise_or)
        nc.vector.tensor_tensor(out=Hd[:, :, dw], in0=Hd[:, :, dw], in1=HR[:, :, dw], op=OP.bitwise_or)
        stt_u32(nc, Vd[:, :, dw], Hd[:, :, dw], 1,
                Hd[:, :, dw], OP.logical_shift_left, OP.bitwise_or)
        stt_u32(nc, Vd[:, :, dw], Hd[:, :, dm], 31,
                Vd[:, :, dw], OP.logical_shift_right, OP.bitwise_or)
        stt_u32(nc, Vd[:, :, dw], Hd[:, :, dw], 1,
                Vd[:, :, dw], OP.logical_shift_right, OP.bitwise_or)
        stt_u32(nc, Vd[:, :, dw], Hd[:, :, dp], 31,
                Vd[:, :, dw], OP.logical_shift_left, OP.bitwise_or)
        nc.vector.tensor_tensor(out=ta[:, :, dw], in0=Ww[:, :, dw], in1=Vd[:, :, dw], op=OP.bitwise_and)
        nc.vector.tensor_tensor(out=Sw[:, :, dw], in0=Sw[:, :, dw], in1=ta[:, :, dw], op=OP.bitwise_or)

    def make_pack(pool):
        pk_l1 = pool.tile([128, NCB, 272], BF16, name="l1", tag="l1")
        pk_l2 = pool.tile([128, NCB, 136], BF16, name="l2", tag="l2")
        pk_l3 = pool.tile([128, NCB, 68], BF16, name="l3", tag="l3")
        pk_li = pool.tile([128, NCB, 34], U32, name="li", tag="li")
        pk_lsh = pool.tile([128, NCB, 17], U32, name="lsh", tag="lsh")
        return pk_l1, pk_l2, pk_l3, pk_li, pk_lsh

    def pack_part(pk, eng, srcf, dstw, w0, w1):
        l1, l2, l3, li, lsh = pk
        s_hi = srcf[:, :, 2 + 32 * w0:2 + 32 * w1:2]
        s_lo = srcf[:, :, 1 + 32 * w0:1 + 32 * w1:2]
        l1w = l1[:, :, 16 * w0:16 * w1]
        l1r = (l1[:, :, 16 * w0 + 1:16 * w1:2], l1[:, :, 16 * w0:16 * w1 - 1:2])
        l2w = l2[:, :, 8 * w0:8 * w1]
        l2r = (l2[:, :, 8 * w0 + 1:8 * w1:2], l2[:, :, 8 * w0:8 * w1 - 1:2])
        l3w = l3[:, :, 4 * w0:4 * w1]
        l3r = (l3[:, :, 4 * w0 + 1:4 * w1:2], l3[:, :, 4 * w0:4 * w1 - 1:2])
        liw = li[:, :, 2 * w0:2 * w1]
        lshw = lsh[:, :, w0:w1]
        lshr = (li[:, :, 2 * w0 + 1:2 * w1:2], li[:, :, 2 * w0:2 * w1 - 1:2])
        dw = dstw[:, :, 1 + w0:1 + w1]
        eng.scalar_tensor_tensor(out=l1w, in0=s_hi, scalar=2.0, in1=s_lo, op0=OP.mult, op1=OP.add)
        eng.scalar_tensor_tensor(out=l2w, in0=l1r[0], scalar=4.0, in1=l1r[1], op0=OP.mult, op1=OP.add)
        eng.scalar_tensor_tensor(out=l3w, in0=l2r[0], scalar=16.0, in1=l2r[1], op0=OP.mult, op1=OP.add)
        eng.scalar_tensor_tensor(out=liw, in0=l3r[0], scalar=256.0, in1=l3r[1], op0=OP.mult, op1=OP.add)
        nc.vector.tensor_scalar(out=lshw, in0=lshr[0], scalar1=16, scalar2=None, op0=OP.logical_shift_left)
        nc.vector.tensor_tensor(out=dw, in0=lshr[1], in1=lshw, op=OP.bitwise_or)
    es_mask = ctx.enter_context(ExitStack())
    maskp = es_mask.enter_context(tc.tile_pool(name="maskp", bufs=1))
    maskH = maskp.tile([128, NCB, RD], MASK_DT, name="maskH")
    maskV = maskp.tile([128, NCB, RD], MASK_DT, name="maskV")
    pmask = maskp.tile([128, NCB, RD], MASK_DT, name="pmask")

    with tc.tile_pool(name="gvt", bufs=1) as gvtp:
        gvT = [gvtp.tile([128, NCB, RD], F32, name=f"gvT{i}", tag=f"gvT{i}") for i in range(2)]
        wh_s = gvtp.tile([128, NHS, 128], F32)
        nc.sync.dma_start(wh_s[:], wh_in)
        whx_s = gvtp.tile([64, NCB, 128], F32)
        nc.sync.dma_start(whx_s[:], whx_in)
        gvXs = gvtp.tile([64, NCB, 194], F32, name="gvXs")
        nc.gpsimd.memset(gvXs[:], 0.0)
        # ---- vertical convs in two passes, H chunk 0 interleaved between ----
        es_x = ExitStack()
        xp = es_x.enter_context(tc.tile_pool(name="xp", bufs=1))
        gr = es_x.enter_context(tc.tile_pool(name="gr", bufs=2))
        wv_s = xp.tile([128, NVS, 128], F32)
        nc.sync.dma_start(wv_s[:], wv_in)
        xtiles = {}

        def load_chunk(S):
            nrows = VKS[S]
            g = xp.tile([128, 1024], F32, name=f"gray{S}", tag=f"gray{S % 2}")
            for h in range(2):
                cs = slice(512 * h, 512 * h + 512)
                t = xp.tile([128, 3, 512], F32, name=f"xs{S}{h}", tag=f"xs{h}")
                nc.sync.dma_start(
                    out=t[0:nrows, :, :],
                    in_=_ap(x_in, 122 * S * 1024 + 512 * h,
                            [[1024, nrows], [XP * 1024, 3], [1, 512]]))
                nc.vector.scalar_tensor_tensor(out=g[0:nrows, cs], in0=t[0:nrows, 0, :],
                                               scalar=float(np.float32(0.299) / np.float32(0.587)),
                                               in1=t[0:nrows, 1, :], op0=OP.mult, op1=OP.add)
                nc.vector.scalar_tensor_tensor(out=g[0:nrows, cs], in0=g[0:nrows, cs],
                                               scalar=float(np.float32(0.587) / np.float32(0.114)),
                                               in1=t[0:nrows, 2, :], op0=OP.mult, op1=OP.add)
            xtiles[S] = g

        def vconv_pass(Ts, pvp, ptp):
            for T in Ts:
                load_chunk(T)
                BT, K = VBKS[T], VKS[T]
                for ci in range(2):
                    ps = pvp.tile([128, 1024], F32, name=f"pv{T}{ci}", tag="pv")
                    for nh in range(2):
                        cols = slice(512 * nh, 512 * nh + 512)
                        nc.tensor.matmul(ps[0:BT, cols], wv_s[0:K, ci * NT + T, 0:BT],
                                         xtiles[T][0:K, cols], start=True, stop=True)
                    grm = gr.tile([128, 1024], F32, name=f"grm{T}{ci}", tag="grm")
                    if ci == 0:
                        nc.scalar.copy(grm[0:BT, :], ps[0:BT, :])
                    else:
                        nc.vector.tensor_copy(grm[0:BT, :], ps[0:BT, :])
                    for g in range(2):
                        ptile = ptp.tile([128, 512], F32, name=f"pt{ci}{T}{g}", tag="pt")
                        for k in range(4):
                            cb = 4 * g + k
                            nc.tensor.transpose(
                                ptile[0:128, 128 * k:128 * k + BT],
                                grm[0:BT, 128 * cb:128 * cb + 128],
                                ident[0:BT, 0:BT].bitcast(F32))
                        nc.scalar.copy(
                            gvT[ci][:, 4 * g:4 * g + 4, 1 + 122 * T:1 + 122 * T + BT],
                            ptile[:].rearrange("p (c b) -> p c b", c=4)[:, :, 0:BT])

        with tc.tile_pool(name="pva", bufs=2, space="PSUM") as pva, \
             tc.tile_pool(name="pta", bufs=3, space="PSUM") as pta:
            vconv_pass([0, 1, 2], pva, pta)
        if STAGE < 2:
            return
        # ---- scope 3: horizontal convs + masks + m2, per-chunk NMS overlapped ----
        NCH = NCH4
        t2c = float(np.float32(TAN225) * np.float32(TAN225))
        SM2, WM2 = thresh2(0.2), thresh2(0.1)
        with tc.tile_pool(name="nmsa", bufs=1) as na, \
             tc.tile_pool(name="nmsb", bufs=1) as nb, \
             tc.tile_pool(name="ph", bufs=2, space="PSUM") as ph:

            sqtiles = {}

            def halo_fill(ic):
                lo, hi = RCH[ic]
                CN = hi - lo
                for ci in range(2):
                    b = 32 * ci
                    # rows b+0:3 <- input cols 128cb-3..-1; rows b+3:6 <- cols 128cb+128..+130
                    nc.sync.dma_start(out=gvXs[b:b + 3, 1:8, 0:CN],
                                      in_=gvT[ci][125:128, 0:7, lo:hi])
                    nc.scalar.dma_start(out=gvXs[b + 3:b + 6, 0:7, 0:CN],
                                        in_=gvT[ci][0:3, 1:8, lo:hi])

            def conv_mm(ic):
                lo, hi = RCH[ic]
                CN = hi - lo
                for cb in range(NCB):
                    pg = [ph.tile([128, CN], F32, name=f"pg{i}", tag=f"pg{i}") for i in range(2)]
                    for ci in range(2):
                        s = ci * NCB + cb
                        nc.tensor.matmul(pg[ci][:, 0:CN], wh_s[0:128, s, 0:128],
                                         gvT[ci][0:128, cb, lo:hi], start=True, stop=False)
                        b = 32 * ci
                        nc.tensor.matmul(pg[ci][:, 0:CN], whx_s[b:b + 6, cb, 0:128],
                                         gvXs[b:b + 6, cb, 0:CN], start=False, stop=True)
                    sqx = na.tile([128, CN], F32, name=f"sqx{ic}{cb}", tag=f"sqx{cb}")
                    sqy = na.tile([128, CN], F32, name=f"sqy{ic}{cb}", tag=f"sqy{cb}")
                    gyc = na.tile([128, CN], F32, name=f"gyc{ic}{cb}", tag=f"gyc{cb}")
                    nc.scalar.activation(sqx[:], pg[0][:, 0:CN], ACT.Square)
                    nc.scalar.activation(sqy[:], pg[1][:, 0:CN], ACT.Square)
                    nc.scalar.copy(gyc[:], pg[1][:, 0:CN])
                    nc.vector.tensor_tensor(out=gyc[:].bitcast(U32), in0=pg[0][:, 0:CN].bitcast(U32),
                                            in1=gyc[:].bitcast(U32), op=OP.bitwise_xor)
                    nc.gpsimd.tensor_tensor(out=magb[:, cb, lo:hi], in0=sqx[:], in1=sqy[:], op=OP.add)
                    sqtiles[(ic, cb)] = (sqx, sqy, gyc[:].bitcast(U32))

            def conv_post(ic):
                lo, hi = RCH[ic]
                for cb in range(NCB):
                    sqx, sqy, xr = sqtiles[(ic, cb)]
                    nc.vector.tensor_scalar(out=pmask[:, cb, lo:hi], in0=xr,
                                            scalar1=2147483648, scalar2=None, op0=OP.is_lt)
                    nc.vector.scalar_tensor_tensor(out=maskH[:, cb, lo:hi], in0=sqx[:],
                                                   scalar=t2c, in1=sqy[:],
                                                   op0=OP.mult, op1=OP.is_ge)
                    nc.vector.scalar_tensor_tensor(out=maskV[:, cb, lo:hi], in0=sqy[:],
                                                   scalar=t2c, in1=sqx[:],
                                                   op0=OP.mult, op1=OP.is_gt)

            def shift_dmas(jc):
                nlo, nhi = NCH[jc]
                ra, rb = nlo - 1, nhi + 1
                CNH = rb - ra
                nc.sync.dma_start(out=m2L[jc][1:128, :, 0:CNH], in_=magb[0:127, :, ra:rb])
                nc.sync.dma_start(out=m2L[jc][0:1, 1:8, 0:CNH], in_=magb[127:128, 0:7, ra:rb])
                nc.scalar.dma_start(out=m2R[jc][0:127, :, 0:CNH], in_=magb[1:128, :, ra:rb])
                nc.scalar.dma_start(out=m2R[jc][127:128, 0:7, 0:CNH], in_=magb[0:1, 1:8, ra:rb])

            def nms_chunk(jc):
                nlo, nhi = NCH[jc]
                CN = nhi - nlo
                ra = nlo - 1
                c0 = slice(nlo, nhi)
                cm = slice(nlo - 1, nhi - 1)
                cp_ = slice(nlo + 1, nhi + 1)
                lc0 = slice(1, 1 + CN)
                lcm = slice(0, CN)
                lcp = slice(2, 2 + CN)
                L, R = m2L[jc], m2R[jc]
                nbsel_t = nb.tile([128, NCB, CN], F32, name=f"nbsel{jc}", tag="nbsel")
                tmp_t = nb.tile([128, NCB, CN], F32, name=f"tmp{jc}", tag="tmp")
                nbsel, tmp = nbsel_t[:], tmp_t[:]
                msk = nbsel
                nc.vector.tensor_tensor(out=nbsel, in0=L[:, :, lcp], in1=R[:, :, lcm], op=OP.max)
                nc.vector.tensor_tensor(out=tmp, in0=R[:, :, lcp], in1=L[:, :, lcm], op=OP.max)
                nc.vector.copy_predicated(nbsel, pmask[:, :, c0], tmp)
                nc.vector.tensor_tensor(out=tmp, in0=magb[:, :, cm], in1=magb[:, :, cp_], op=OP.max)
                nc.vector.copy_predicated(nbsel, maskV[:, :, c0], tmp)
                nc.vector.tensor_tensor(out=tmp, in0=L[:, :, lc0], in1=R[:, :, lc0], op=OP.max)
                nc.vector.copy_predicated(nbsel, maskH[:, :, c0], tmp)
                nc.vector.tensor_tensor(out=tmp, in0=magb[:, :, c0], in1=nbsel, op=OP.is_gt)
                nc.vector.scalar_tensor_tensor(out=msk, in0=magb[:, :, c0], scalar=1e-6,
                                               in1=tmp, op0=OP.add, op1=OP.mult)
                nc.vector.tensor_scalar(out=smb[:, :, c0], in0=msk,
                                        scalar1=SM2, scalar2=None, op0=OP.is_gt)
                nc.vector.tensor_scalar(out=wmb[:, :, c0], in0=msk,
                                        scalar1=WM2, scalar2=None, op0=OP.is_gt)
                nc.scalar.activation(magob[:, :, c0], msk, ACT.Sqrt)

            pk = make_pack(nb)
            halo_fill(0)
            conv_mm(0)
            with tc.tile_pool(name="pvb", bufs=1, space="PSUM") as pvb, \
                 tc.tile_pool(name="ptb", bufs=2, space="PSUM") as ptb:
                vconv_pass([3, 4], pvb, ptb)
            conv_post(0)
            shift_dmas(0)
            nms_chunk(0)
            pack_part(pk, nc.vector, smb, Sw, *PKW[0])
            pack_part(pk, nc.vector, wmb, Ww, *PKW[0])
            for jc in (1, 2):
                halo_fill(jc)
                conv_mm(jc)
                conv_post(jc)
                shift_dmas(jc)
                nms_chunk(jc)
                if jc == 2:
                    flood_iter(slice(1, 11))   # early iter 0 on packed words; stale-zero seam
                pack_part(pk, nc.vector, smb, Sw, *PKW[jc])
                pack_part(pk, nc.vector, wmb, Ww, *PKW[jc])
        es_x.close()
    es_mask.close()
    if STAGE < 4:
        return
    # ---- scope 4: flood fill, unpack, transpose-out, DMA out ----
    if True:
        es_out = ExitStack()
        stp = es_out.enter_context(tc.tile_pool(name="st", bufs=1))
        pto = es_out.enter_context(tc.tile_pool(name="pto", bufs=4, space="PSUM"))
        def emit_output(oi, src_t, dst, r0off, bf=False):
            stage = stp.tile([128, NT, 1024], F32, name=f"stage{oi}", tag="stage")
            for T in range(NT):
                BT = BTS[T]
                for g in range(2):
                    ptile = pto.tile([128, 512], BF16 if bf else F32, name=f"pto{oi}{T}{g}", tag="pto")
                    for k in range(4):
                        cb = 4 * g + k
                        if bf:
                            nc.tensor.transpose(
                                ptile[0:BT, 128 * k:128 * k + 128],
                                src_t[:, cb, r0off + 128 * T: r0off + 128 * T + BT],
                                identb[:])
                        else:
                            nc.tensor.transpose(
                                r32(ptile[0:BT, 128 * k:128 * k + 128]),
                                r32(src_t[:, cb, r0off + 128 * T: r0off + 128 * T + BT]),
                                ident[:])
                    nc.scalar.copy(stage[0:BT, T, 512 * g:512 * g + 512], ptile[0:BT, :])
            for T in range(4):
                nc.sync.dma_start(
                    out=_ap(dst, 128 * T * 1024, [[1024, 128], [1, 1024]]),
                    in_=stage[:, T, :])
            nc.sync.dma_start(
                out=_ap(dst, 512 * 1024, [[1024, 17], [1, 1024]]),
                in_=stage[0:17, 4, :])
        emit_output(0, magob, mag_out, 1, bf=True)
        for it in range(1, ITERS):
            flood_iter(slice(1, 18))

        if STAGE < 6:
            return
        edgesT = stp.tile([128, NCB, PACK_ROWS], F32R, tag="edgesT")
        eti = stp.tile([128, NCB, PACK_ROWS], U32, tag="eti")
        stage = stp.tile([128, NT, 1024], F32, name="stageE", tag="stage")
        pap = pat_s[:, :]
        for T in range(NT):
            BT = BTS[T]
            W = 4 if T < 4 else 1
            sap = Sw[:, :, 1 + 4 * T:1 + 4 * T + W]
            bits_in = bass.AP(sap.tensor, sap.offset, list(sap.ap) + [[0, 32]])
            pat_bc = bass.AP(pap.tensor, pap.offset, [list(pap.ap[0]), [0, NCB], [0, W], list(pap.ap[1])])
            rsl = slice(128 * T, 128 * T + 32 * W)
            nc.vector.tensor_tensor(out=eti[:, :, rsl].rearrange("p c (j k) -> p c j k", k=32),
                                    in0=bits_in, in1=pat_bc, op=OP.bitwise_and)
            nc.vector.tensor_scalar(out=edgesT[:, :, rsl], in0=eti[:, :, rsl],
                                    scalar1=0, scalar2=None, op0=OP.not_equal)
            for g in range(2):
                ptile = pto.tile([128, 512], F32, name=f"ptoE{T}{g}", tag="pto")
                for k in range(4):
                    cb = 4 * g + k
                    nc.tensor.transpose(
                        r32(ptile[0:BT, 128 * k:128 * k + 128]),
                        r32(edgesT[:, cb, 128 * T:128 * T + BT]),
                        ident[:])
                nc.scalar.copy(stage[0:BT, T, 512 * g:512 * g + 512], ptile[0:BT, :])
            nc.sync.dma_start(
                out=_ap(edges_out, 128 * T * 1024, [[1024, BT], [1, 1024]]),
                in_=stage[0:BT, T, :])
        es_out.close()


_CACHE = {}

def _build():
    if 'nc' in _CACHE:
        return
    import concourse.bacc as bacc
    import concourse.mybir as mybir
    import concourse.tile as tile
    from contextlib import ExitStack
    patT, wvT, whT, whxT, metaT = make_core_inputs(True)
    patB, wvB, whB, whxB, metaB = make_core_inputs(False)
    assert metaT == metaB
    nc = bacc.Bacc("TRN2", target_bir_lowering=False, debug=False)
    x = nc.dram_tensor("x", [3, XP, 1024], mybir.dt.float32, kind="ExternalInput")
    wv = nc.dram_tensor("wv", list(wvT.shape), mybir.dt.float32, kind="ExternalInput")
    wh = nc.dram_tensor("wh", list(whT.shape), mybir.dt.float32, kind="ExternalInput")
    whx = nc.dram_tensor("whx", list(whxT.shape), mybir.dt.float32, kind="ExternalInput")
    pat = nc.dram_tensor("pat", [128, 32], mybir.dt.uint32, kind="ExternalInput")
    identt = nc.dram_tensor("ident", [128, 128], mybir.dt.float32r, kind="ExternalInput")
    mag_o = nc.dram_tensor("mag_o", [529, 1024], mybir.dt.float32, kind="ExternalOutput")
    edges_o = nc.dram_tensor("edges_o", [529, 1024], mybir.dt.float32, kind="ExternalOutput")
    with ExitStack() as ctx:
        tc = ctx.enter_context(tile.TileContext(nc))
        canny_core(ctx, tc, [mag_o.ap(), edges_o.ap()],
                   [x.ap(), wv.ap(), wh.ap(), whx.ap(), pat.ap(), identt.ap()], metaT)
    nc.finalize()
    _CACHE.update(nc=nc, weights=dict(top=(patT, wvT, whT, whxT), bot=(patB, wvB, whB, whxB)))

def kernel(x):
    _build()
    from concourse.bass_utils import run_bass_kernel_spmd
    nc = _CACHE['nc']
    x = np.ascontiguousarray(np.asarray(x, dtype=np.float32))
    B = x.shape[0]
    in_maps = []
    for core in range(8):
        b, half = core // 2, core % 2
        top = (half == 0)
        pad = np.zeros((3, 3, 1024), np.float32)
        if top:
            xw = np.concatenate([pad, x[b, :, 0:532, :]], axis=1)
        else:
            xw = np.concatenate([x[b, :, 492:1024, :], pad], axis=1)
        patc, wvc, whc, whxc = _CACHE['weights']['top' if top else 'bot']
        in_maps.append({"x": np.ascontiguousarray(xw), "wv": wvc, "wh": whc, "whx": whxc,
                        "pat": patc, "ident": np.eye(128, dtype=np.float32)})
    res = run_bass_kernel_spmd(nc, in_maps, core_ids=list(range(8)))
    mag = np.zeros((B, 1, 1024, 1024), np.float32)
    edges = np.zeros((B, 1, 1024, 1024), np.float32)
    for core in range(8):
        b, half = core // 2, core % 2
        r = res.results[core]
        if half == 0:
            mag[b, 0, 0:512] = r["mag_o"][0:512]
            edges[b, 0, 0:512] = r["edges_o"][0:512]
        else:
            mag[b, 0, 512:1024] = r["mag_o"][17:529]
            edges[b, 0, 512:1024] = r["edges_o"][17:529]
    return mag, edges



# revision 47
# speedup vs baseline: 1.0202x; 1.0202x over previous
"""Self-contained Trainium2 Bass kernel for Canny edge detection (4,3,1024,1024).

kernel(x) -> (magnitude, edges), each [4,1,1024,1024] f32. 8 NeuronCores SPMD:
core = (batch, image half); no cross-core communication (flood-fill halo margin).
"""
import numpy as np

XR = 532          # x window rows per core
NM = 529          # mag rows per core
H_IMG, W_IMG = 1024, 1024
RD = 532          # r-dim of mag-grid col-major buffers: slot = 1+M, guards at 0,530
WSLOT = 19        # flood word slots: 0 guard, 1..17 data, 18 guard
PACK_ROWS = 544   # 17 words * 32 rows
ITERS = 4
GRAY_W = np.array([0.299, 0.587, 0.114], np.float32)
TAN225 = np.float32(np.tan(np.pi / 8))  # 0.41421356

def thresh2(c):
    """Largest f32 v* with (v > v*) == (f32(sqrt(v)) > c) for f32 v; NMS thresholds on m2+eps."""
    c = np.float32(c)
    v = np.float32(c * c)
    while np.float32(np.sqrt(v)) > c:
        v = np.nextafter(v, np.float32(0), dtype=np.float32)
    while np.float32(np.sqrt(np.nextafter(v, np.float32(np.inf), dtype=np.float32))) <= c:
        v = np.nextafter(v, np.float32(np.inf), dtype=np.float32)
    return float(v)

def gauss5():
    # f32 replica of reference._gaussian_kernel1d(5, 1.0)
    x = (np.arange(5, dtype=np.float32) - 2).astype(np.float32)
    g = np.exp((-x * x / np.float32(2.0)).astype(np.float32)).astype(np.float32)
    return (g / g.sum(dtype=np.float32)).astype(np.float32)

def _op_conv(n_out, n_in, taps, center, idx_map):
    """Row t of output = sum_d taps[d] * in[idx_map(t + d - center)], f64 build."""
    C = np.zeros((n_out, n_in), np.float64)
    for t in range(n_out):
        for d, w in enumerate(taps):
            s = idx_map(t + d - center)
            C[t, s] += w
    return C

def reflect_idx(i, n):
    # jnp.pad 'reflect': -1 -> 1, -2 -> 2; n -> n-2, n+1 -> n-3
    if i < 0:
        return -i
    if i >= n:
        return 2 * n - 2 - i
    return i

def clamp_idx(i, n):
    return min(max(i, 0), n - 1)

def build_vertical_ops(top: bool):
    """Return (Cvx, Cvy): [NM, XR] f32 composed vertical operators for this core."""
    g = gauss5().astype(np.float64)
    # Stage 1: gauss vertical with reflect at IMAGE edges, over x window rows.
    # blurred-v needed rows: image rows of M-1 .. M+1 -> local B = -1..529
    # local->img: top: img = local_x;  bottom: img = 492 + local_x
    # blurV local grid b = -1..529 maps to img rows (top: b, bottom: 492+b... wait
    #   bottom mag M -> img 495+M; blur rows needed img 494..1024)
    # Build on local-x axis directly with the correct edge behavior:
    #   top: local 0 == img 0 (reflect boundary at local 0); far end interior.
    #   bottom: local 531 == img 1023 (reflect boundary there); near end interior.
    NB = 531  # blur rows b = -1..529 stored t = b+1
    def xmap_top(i):   # reflect at 0 only (other end never reached out of range)
        return reflect_idx(i, 10**9) if i >= 0 else -i
    def xmap_bot(i):
        if i >= XR:
            return 2 * XR - 2 - i
        return i
    xmap = xmap_top if top else xmap_bot
    # blur b (local-x coordinate of output): top: b = t-1; bottom: b = t-1+2
    #   top: blurV[b] centered at x local row b;  b from -1..529
    #   bottom: mag M -> img 495+M -> local x = 495+M-492 = 3+M; blur rows local x = 2+M-1.. 
    #     blur grid b(local x) = 2 .. 532 for t=0..530
    off = 0 if top else 3
    Cb = np.zeros((NB, XR), np.float64)
    for t in range(NB):
        b = t - 1 + off   # local-x row this blurV output is centered on
        for d in range(5):
            s = b + d - 2
            s = xmap(s)
            assert 0 <= s < XR, (top, t, s)
            Cb[t, s] += g[d]
    # Stage 2: sobel vertical ops on the blurV grid with replicate at IMAGE edges.
    # mag M: taps at blur rows b = (M-1 .. M+1) in local-x => stored t = M-1+1..=M..M+2 - wait
    #   stored t of blur row b: t = b + 1 - off ... b_local_x = M + off + db where db=-1..1
    #   stored t = (M + off + db) - off + 1 - ... let me just: stored t corresponds to b_lx = t-1+off
    #   For mag M: need b_lx = (M+off) + db  => t = M + 1 + db
    # replicate at image edges: top: b_lx < 0 -> 0 i.e. t<0 -> t=0? replicate on blur IMG rows:
    #   top: blur img row = b_lx; replicate row<0 -> row 0 -> t index of b_lx=0 is t=1.
    #   bottom: blur img row = 492 + b_lx; replicate row>1023 -> b_lx>531 -> clamp to 531 (t=530)
    vsm = np.array([1.0, 2.0, 1.0])
    vdf = np.array([-1.0, 0.0, 1.0])
    Cvx = np.zeros((NM, XR), np.float64)
    Cvy = np.zeros((NM, XR), np.float64)
    for M in range(NM):
        for db, (wx, wy) in enumerate(zip(vsm, vdf)):
            t = M + db  # t = M+1+(db-1)
            if top:
                t = max(t, 1)       # replicate img row 0 (t=1)... t=0 is b_lx=-1 (img -1)
            else:
                t = min(t, NB - 2)  # replicate img row 1023 at far end (t=529)
            # also clamp other end (never used beyond range by construction)
            t = min(max(t, 0), NB - 1)
            Cvx[M] += wx * Cb[t]
            Cvy[M] += wy * Cb[t]
    return Cvx.astype(np.float32), Cvy.astype(np.float32)

def build_horizontal_ops():
    """(Chx, Chy): [W, W] composed horizontal operators (same both cores)."""
    g = gauss5().astype(np.float64)
    Cb = _op_conv(W_IMG, W_IMG, g, 2, lambda i: reflect_idx(i, W_IMG))
    Dif = _op_conv(W_IMG, W_IMG, [-1.0, 0.0, 1.0], 1, lambda i: clamp_idx(i, W_IMG))
    Sm = _op_conv(W_IMG, W_IMG, [1.0, 2.0, 1.0], 1, lambda i: clamp_idx(i, W_IMG))
    Chx = (Dif @ Cb).astype(np.float32)
    Chy = (Sm @ Cb).astype(np.float32)
    return Chx, Chy

# ---------------- numpy model of the per-core pipeline (for validation) -------------
def core_model(x_win, top):
    """x_win: [3, XR, 1024] f32. Returns (magout [NM,1024], edges [NM,1024])."""
    Cvx, Cvy = build_vertical_ops(top)
    Chx, Chy = build_horizontal_ops()
    gray = np.tensordot(GRAY_W, x_win.astype(np.float32), 1)  # [XR, W]
    gvx = (Cvx @ gray).astype(np.float32)
    gvy = (Cvy @ gray).astype(np.float32)
    gx = (gvx @ Chx.T).astype(np.float32)
    gy = (gvy @ Chy.T).astype(np.float32)
    m2 = gx * gx + gy * gy
    mag = np.sqrt(m2 + np.float32(1e-6)).astype(np.float32)
    magp = np.zeros((NM + 2, W_IMG + 2), np.float32)
    magp[1:-1, 1:-1] = mag
    ax, ay = np.abs(gx), np.abs(gy)
    maskH = (TAN225 * ax) >= ay
    maskV = (TAN225 * ay) > ax
    pmask = (gx * gy) >= 0
    c = magp[1:-1, 1:-1]
    up, dn = magp[0:-2, 1:-1], magp[2:, 1:-1]
    lf, rt = magp[1:-1, 0:-2], magp[1:-1, 2:]
    ul, ur = magp[0:-2, 0:-2], magp[0:-2, 2:]
    dl, dr = magp[2:, 0:-2], magp[2:, 2:]
    nbH = np.maximum(lf, rt); nbV = np.maximum(up, dn)
    nbD1 = np.maximum(dr, ul); nbD2 = np.maximum(dl, ur)
    nbsel = nbD2.copy()
    nbsel[pmask] = nbD1[pmask]
    nbsel[maskV] = nbV[maskV]
    nbsel[maskH] = nbH[maskH]
    ismax = c > nbsel
    magout = mag * ismax
    sm = magout > np.float32(0.2)
    wm = magout > np.float32(0.1)
    S = sm.copy(); W = wm
    for _ in range(ITERS):
        Sp = np.zeros((NM + 2, W_IMG + 2), bool)
        Sp[1:-1, 1:-1] = S
        dil = Sp[0:-2,0:-2]|Sp[0:-2,1:-1]|Sp[0:-2,2:]|Sp[1:-1,0:-2]|Sp[1:-1,1:-1]|Sp[1:-1,2:]|Sp[2:,0:-2]|Sp[2:,1:-1]|Sp[2:,2:]
        S = S | (W & dil)
    return magout, S.astype(np.float32)


import numpy as np
from collections import defaultdict
import concourse.bass as bass
import concourse.mybir as mybir
from concourse.masks import make_identity


F32, I32, U32, U8 = mybir.dt.float32, mybir.dt.int32, mybir.dt.uint32, mybir.dt.uint8
F32R = mybir.dt.float32r
BF16 = mybir.dt.bfloat16
OP = mybir.AluOpType
ACT = mybir.ActivationFunctionType
MASK_DT = U8

NT = 5
BTS = [128, 128, 128, 128, 17]
NCB = 8
RCH = [(1, 162), (162, 354), (354, 530)]     # conv r-slot chunks
NCH4 = [(1, 161), (161, 353), (353, 530)]    # NMS r-slot chunks (word-aligned)
PKW = [(0, 5), (5, 11), (11, 17)]            # pack word ranges per NMS chunk

XP = 535           # padded x rows: top = [0,0,0, img 0..531]; bottom = [img 492..1023, 0,0,0]
VBKS = [122, 122, 122, 122, 41]   # output rows per vertical block (sum 529)
VKS = [128, 128, 128, 128, 47]    # input rows per block, start = 122*k

def build_vplan(top):
    Cvx, Cvy = build_vertical_ops(top)
    w = np.float64(np.float32(GRAY_W[2]))  # 0.114 folded out of the DVE gray stage
    # pad to the unified 535-row local axis
    pads = []
    for C in (Cvx, Cvy):
        Cp = np.zeros((NM, XP), np.float64)
        if top:
            Cp[:, 3:3 + XR] = C
        else:
            Cp[:, 0:XR] = C
        pads.append(Cp)
    arr = np.zeros((128, 2 * NT, 128), np.float32)
    for ci, C in enumerate(pads):
        for k in range(NT):
            r0, BK, K = 122 * k, VBKS[k], VKS[k]
            sub = C[r0:r0 + BK, :]
            assert np.all(sub[:, :122 * k] == 0) and np.all(sub[:, 122 * k + K:] == 0), (top, ci, k)
            arr[0:K, ci * NT + k, 0:BK] = (w * sub[:, 122 * k:122 * k + K]).T
    return arr

def build_hplan():
    """wh [128, 16, 128]: slot ci*8+cb = dense diagonal block (input cols 128cb..+127).
    whx [8, 16, 128]: halo block: rows 0..2 = input cols 128cb-3..-1, rows 3..5 = 128cb+128..+130."""
    Chx, Chy = build_horizontal_ops()
    wh = np.zeros((128, 2 * NCB, 128), np.float32)
    whx = np.zeros((64, NCB, 128), np.float32)
    for ci, C in enumerate((Chx, Chy)):
        for cb in range(NCB):
            p0 = 128 * cb
            s = ci * NCB + cb
            b = 32 * ci
            wh[:, s, :] = C[p0:p0 + 128, p0:p0 + 128].T
            if cb > 0:
                whx[b:b + 3, cb, :] = C[p0:p0 + 128, p0 - 3:p0].T
            if cb < NCB - 1:
                whx[b + 3:b + 6, cb, :] = C[p0:p0 + 128, p0 + 128:p0 + 131].T
            assert np.all(C[p0:p0 + 128, :max(p0 - 3, 0)] == 0)
            assert np.all(C[p0:p0 + 128, p0 + 131:] == 0)
    return wh, whx

def pack_blocks(blocks, kinds):
    """kinds[i] in {'full','lo32','hi32'}; hi32 must land at k0=96, lo32/full at 0."""
    places = [None] * len(blocks)
    slots = []
    free_lo, free_hi = [], []
    for i, (b, kind) in enumerate(zip(blocks, kinds)):
        if kind == 'full':
            slots.append([])
            slots[-1].append((0, b))
            places[i] = (len(slots) - 1, 0)
        elif kind == 'lo32':
            if not free_lo:
                slots.append([])
                free_hi.append(len(slots) - 1)
                free_lo.append(len(slots) - 1)
            s = free_lo.pop(0)
            slots[s].append((0, b))
            places[i] = (s, 0)
        else:  # hi64 at k0=64
            if not free_hi:
                slots.append([])
                free_lo.append(len(slots) - 1)
                free_hi.append(len(slots) - 1)
            s = free_hi.pop(0)
            slots[s].append((64, b))
            places[i] = (s, 64)
    arr = np.zeros((128, len(slots), 128), np.float32)
    for slot, entries in enumerate(slots):
        for k0, b in entries:
            K, M = b.shape
            arr[k0:k0 + K, slot, 0:M] = b
    return arr, places

def make_core_inputs(top):
    wv = build_vplan(top)
    wh, whx = build_hplan()
    pat = np.tile(np.uint32(1) << np.arange(32, dtype=np.uint32), (128, 1))
    meta = dict(nv=wv.shape[1], nh=wh.shape[1])
    return np.ascontiguousarray(pat), wv, wh, whx, meta

def _ap(base_ap, offset_elems, dims):
    return bass.AP(base_ap.tensor, base_ap.offset + offset_elems, dims)

def r32(ap):
    return ap.bitcast(mybir.dt.float32r)

def stt_u32(nc, out, in0, scalar, in1, op0, op1):
    """scalar_tensor_tensor with an integer (u32) immediate, for bitvec ops."""
    eng = nc.vector
    return eng.add_instruction(
        mybir.InstTensorScalarPtr(
            name=nc.get_next_instruction_name(),
            is_scalar_tensor_tensor=True,
            op0=op0,
            op1=op1,
            ins=[eng.lower_ap(in0),
                 mybir.ImmediateValue(dtype=mybir.dt.uint32, value=scalar),
                 eng.lower_ap(in1)],
            outs=[eng.lower_ap(out)],
        ))

def canny_core(ctx, tc, outs, ins, meta):
    import os
    STAGE = int(os.environ.get('CANNY_STAGE', '9'))
    from contextlib import ExitStack
    nc = tc.nc
    mag_out, edges_out = outs
    x_in, wv_in, wh_in, whx_in, pat_in, ident_in = ins
    NVS, NHS = meta['nv'], meta['nh']

    consts = ctx.enter_context(tc.tile_pool(name="consts", bufs=1))
    pat_s = consts.tile([128, 32], U32)
    nc.sync.dma_start(pat_s[:], pat_in)
    ident = consts.tile([128, 128], F32R)
    nc.sync.dma_start(ident[:], ident_in)
    identb = consts.tile([128, 128], BF16)
    make_identity(nc, identb)

    persist = ctx.enter_context(tc.tile_pool(name="persist", bufs=1))
    magb = persist.tile([128, NCB, RD], F32)          # holds m2 = gx^2+gy^2
    magob = persist.tile([128, NCB, RD], BF16)        # final masked magnitude (bf16 ok: post-decision values)
    nc.gpsimd.memset(magb[:, :, 0:1], 0.0)
    nc.gpsimd.memset(magb[:, :, 530:532], 0.0)
    m2L_s = persist.tile([128, NCB, 194], F32, name="m2L")
    m2R_s = persist.tile([128, NCB, 194], F32, name="m2R")
    m2L = [m2L_s, m2L_s, m2L_s]
    m2R = [m2R_s, m2R_s, m2R_s]
    nc.gpsimd.memset(m2L_s[0:1, 0:1, :], 0.0)
    nc.gpsimd.memset(m2R_s[96:128, 7:8, :], 0.0)

    swp = ctx.enter_context(tc.tile_pool(name="swp", bufs=1))
    smb = swp.tile([128, NCB, PACK_ROWS + 2], U8)
    wmb = swp.tile([128, NCB, PACK_ROWS + 2], U8)
    nc.gpsimd.memset(smb[:, :, 530:546], 0)
    nc.gpsimd.memset(wmb[:, :, 530:546], 0)
    fl = ctx.enter_context(tc.tile_pool(name="fl", bufs=1))
    Sw = fl.tile([128, NCB, WSLOT], U32, tag="Sw")
    Ww = fl.tile([128, NCB, WSLOT], U32, tag="Ww")
    HL = fl.tile([128, NCB, WSLOT], U32, tag="HL")
    HR = fl.tile([128, NCB, WSLOT], U32, tag="HR")
    Hd = fl.tile([128, NCB, WSLOT], U32, tag="Hd")
    Vd = fl.tile([128, NCB, WSLOT], U32, tag="Vd")
    ta = fl.tile([128, NCB, WSLOT], U32, tag="ta")
    for t in (Sw, Ww, HL, HR, Hd, Vd, ta):
        nc.gpsimd.memset(t[:], 0)
    def flood_iter(dw):
        a, b = dw.start, dw.stop
        dm, dp = slice(a - 1, b - 1), slice(a + 1, b + 1)
        nc.sync.dma_start(out=HL[1:128, :, dw], in_=Sw[0:127, :, dw])
        nc.scalar.dma_start(out=HL[0:1, 1:8, dw], in_=Sw[127:128, 0:7, dw])
        nc.gpsimd.dma_start(out=HR[0:127, :, dw], in_=Sw[1:128, :, dw])
        nc.scalar.dma_start(out=HR[127:128, 0:7, dw], in_=Sw[0:1, 1:8, dw])
        nc.vector.tensor_tensor(out=Hd[:, :, dw], in0=Sw[:, :, dw], in1=HL[:, :, dw], op=OP.bitwise_or)
        nc.vector.tensor_tensor(out=Hd[:, :, dw], in0=Hd[:, :, dw], in1=HR[:, :, dw], op=OP.bitwise_or)
        stt_u32(nc, Vd[:, :, dw], Hd[:, :, dw], 1,
                Hd[:, :, dw], OP.logical_shift_left, OP.bitwise_or)
        stt_u32(nc, Vd[:, :, dw], Hd[:, :, dm], 31,
                Vd[:, :, dw], OP.logical_shift_right, OP.bitwise_or)
        stt_u32(nc, Vd[:, :, dw], Hd[:, :, dw], 1,
                Vd[:, :, dw], OP.logical_shift_right, OP.bitwise_or)
        stt_u32(nc, Vd[:, :, dw], Hd[:, :, dp], 31,
                Vd[:, :, dw], OP.logical_shift_left, OP.bitwise_or)
        nc.vector.tensor_tensor(out=ta[:, :, dw], in0=Ww[:, :, dw], in1=Vd[:, :, dw], op=OP.bitwise_and)
        nc.vector.tensor_tensor(out=Sw[:, :, dw], in0=Sw[:, :, dw], in1=ta[:, :, dw], op=OP.bitwise_or)

    def make_pack(pool):
        pk_l1 = pool.tile([128, NCB, 272], BF16, name="l1", tag="l1")
        pk_l2 = pool.tile([128, NCB, 136], BF16, name="l2", tag="l2")
        pk_l3 = pool.tile([128, NCB, 68], BF16, name="l3", tag="l3")
        pk_li = pool.tile([128, NCB, 34], U32, name="li", tag="li")
        pk_lsh = pool.tile([128, NCB, 17], U32, name="lsh", tag="lsh")
        return pk_l1, pk_l2, pk_l3, pk_li, pk_lsh

    def pack_part(pk, eng, srcf, dstw, w0, w1):
        l1, l2, l3, li, lsh = pk
        s_hi = srcf[:, :, 2 + 32 * w0:2 + 32 * w1:2]
        s_lo = srcf[:, :, 1 + 32 * w0:1 + 32 * w1:2]
        l1w = l1[:, :, 16 * w0:16 * w1]
        l1r = (l1[:, :, 16 * w0 + 1:16 * w1:2], l1[:, :, 16 * w0:16 * w1 - 1:2])
        l2w = l2[:, :, 8 * w0:8 * w1]
        l2r = (l2[:, :, 8 * w0 + 1:8 * w1:2], l2[:, :, 8 * w0:8 * w1 - 1:2])
        l3w = l3[:, :, 4 * w0:4 * w1]
        l3r = (l3[:, :, 4 * w0 + 1:4 * w1:2], l3[:, :, 4 * w0:4 * w1 - 1:2])
        liw = li[:, :, 2 * w0:2 * w1]
        lshw = lsh[:, :, w0:w1]
        lshr = (li[:, :, 2 * w0 + 1:2 * w1:2], li[:, :, 2 * w0:2 * w1 - 1:2])
        dw = dstw[:, :, 1 + w0:1 + w1]
        eng.scalar_tensor_tensor(out=l1w, in0=s_hi, scalar=2.0, in1=s_lo, op0=OP.mult, op1=OP.add)
        eng.scalar_tensor_tensor(out=l2w, in0=l1r[0], scalar=4.0, in1=l1r[1], op0=OP.mult, op1=OP.add)
        eng.scalar_tensor_tensor(out=l3w, in0=l2r[0], scalar=16.0, in1=l2r[1], op0=OP.mult, op1=OP.add)
        eng.scalar_tensor_tensor(out=liw, in0=l3r[0], scalar=256.0, in1=l3r[1], op0=OP.mult, op1=OP.add)
        nc.vector.tensor_scalar(out=lshw, in0=lshr[0], scalar1=16, scalar2=None, op0=OP.logical_shift_left)
        nc.vector.tensor_tensor(out=dw, in0=lshr[1], in1=lshw, op=OP.bitwise_or)
    es_mask = ctx.enter_context(ExitStack())
    maskp = es_mask.enter_context(tc.tile_pool(name="maskp", bufs=1))
    maskH = maskp.tile([128, NCB, RD], MASK_DT, name="maskH")
    maskV = maskp.tile([128, NCB, RD], MASK_DT, name="maskV")
    pmask = maskp.tile([128, NCB, RD], MASK_DT, name="pmask")

    with tc.tile_pool(name="gvt", bufs=1) as gvtp:
        gvT = [gvtp.tile([128, NCB, RD], F32, name=f"gvT{i}", tag=f"gvT{i}") for i in range(2)]
        wh_s = gvtp.tile([128, NHS, 128], F32)
        nc.sync.dma_start(wh_s[:], wh_in)
        whx_s = gvtp.tile([64, NCB, 128], F32)
        nc.sync.dma_start(whx_s[:], whx_in)
        gvXs = gvtp.tile([64, NCB, 194], F32, name="gvXs")
        nc.gpsimd.memset(gvXs[:], 0.0)
        # ---- vertical convs in two passes, H chunk 0 interleaved between ----
        es_x = ExitStack()
        xp = es_x.enter_context(tc.tile_pool(name="xp", bufs=1))
        gr = es_x.enter_context(tc.tile_pool(name="gr", bufs=2))
        wv_s = xp.tile([128, NVS, 128], F32)
        nc.sync.dma_start(wv_s[:], wv_in)
        xtiles = {}

        def load_chunk(S):
            nrows = VKS[S]
            g = xp.tile([128, 1024], F32, name=f"gray{S}", tag=f"gray{S % 2}")
            for h in range(2):
                cs = slice(512 * h, 512 * h + 512)
                t = xp.tile([128, 3, 512], F32, name=f"xs{S}{h}", tag=f"xs{h}")
                nc.sync.dma_start(
                    out=t[0:nrows, :, :],
                    in_=_ap(x_in, 122 * S * 1024 + 512 * h,
                            [[1024, nrows], [XP * 1024, 3], [1, 512]]))
                nc.vector.scalar_tensor_tensor(out=g[0:nrows, cs], in0=t[0:nrows, 0, :],
                                               scalar=float(np.float32(0.299) / np.float32(0.587)),
                                               in1=t[0:nrows, 1, :], op0=OP.mult, op1=OP.add)
                nc.vector.scalar_tensor_tensor(out=g[0:nrows, cs], in0=g[0:nrows, cs],
                                               scalar=float(np.float32(0.587) / np.float32(0.114)),
                                               in1=t[0:nrows, 2, :], op0=OP.mult, op1=OP.add)
            xtiles[S] = g

        def vconv_pass(Ts, pvp, ptp):
            for T in Ts:
                load_chunk(T)
                BT, K = VBKS[T], VKS[T]
                for ci in range(2):
                    ps = pvp.tile([128, 1024], F32, name=f"pv{T}{ci}", tag="pv")
                    for nh in range(2):
                        cols = slice(512 * nh, 512 * nh + 512)
                        nc.tensor.matmul(ps[0:BT, cols], wv_s[0:K, ci * NT + T, 0:BT],
                                         xtiles[T][0:K, cols], start=True, stop=True)
                    grm = gr.tile([128, 1024], F32, name=f"grm{T}{ci}", tag="grm")
                    if ci == 0:
                        nc.scalar.copy(grm[0:BT, :], ps[0:BT, :])
                    else:
                        nc.vector.tensor_copy(grm[0:BT, :], ps[0:BT, :])
                    for g in range(2):
                        ptile = ptp.tile([128, 512], F32, name=f"pt{ci}{T}{g}", tag="pt")
                        for k in range(4):
                            cb = 4 * g + k
                            nc.tensor.transpose(
                                ptile[0:128, 128 * k:128 * k + BT],
                                grm[0:BT, 128 * cb:128 * cb + 128],
                                ident[0:BT, 0:BT].bitcast(F32))
                        dst_ap = gvT[ci][:, 4 * g:4 * g + 4, 1 + 122 * T:1 + 122 * T + BT]
                        src_ap = ptile[:].rearrange("p (c b) -> p c b", c=4)[:, :, 0:BT]
                        if ci == 0:
                            nc.scalar.copy(dst_ap, src_ap)
                        else:
                            nc.vector.tensor_copy(dst_ap, src_ap)

        with tc.tile_pool(name="pva", bufs=2, space="PSUM") as pva, \
             tc.tile_pool(name="pta", bufs=3, space="PSUM") as pta:
            vconv_pass([0, 1, 2], pva, pta)
        if STAGE < 2:
            return
        # ---- scope 3: horizontal convs + masks + m2, per-chunk NMS overlapped ----
        NCH = NCH4
        t2c = float(np.float32(TAN225) * np.float32(TAN225))
        SM2, WM2 = thresh2(0.2), thresh2(0.1)
        with tc.tile_pool(name="nmsa", bufs=1) as na, \
             tc.tile_pool(name="nmsb", bufs=1) as nb, \
             tc.tile_pool(name="ph", bufs=2, space="PSUM") as ph:

            sqtiles = {}

            def halo_fill(ic):
                lo, hi = RCH[ic]
                CN = hi - lo
                for ci in range(2):
                    b = 32 * ci
                    # rows b+0:3 <- input cols 128cb-3..-1; rows b+3:6 <- cols 128cb+128..+130
                    nc.sync.dma_start(out=gvXs[b:b + 3, 1:8, 0:CN],
                                      in_=gvT[ci][125:128, 0:7, lo:hi])
                    nc.scalar.dma_start(out=gvXs[b + 3:b + 6, 0:7, 0:CN],
                                        in_=gvT[ci][0:3, 1:8, lo:hi])

            def conv_mm(ic):
                lo, hi = RCH[ic]
                CN = hi - lo
                for cb in range(NCB):
                    pg = [ph.tile([128, CN], F32, name=f"pg{i}", tag=f"pg{i}") for i in range(2)]
                    for ci in range(2):
                        s = ci * NCB + cb
                        nc.tensor.matmul(pg[ci][:, 0:CN], wh_s[0:128, s, 0:128],
                                         gvT[ci][0:128, cb, lo:hi], start=True, stop=False)
                        b = 32 * ci
                        nc.tensor.matmul(pg[ci][:, 0:CN], whx_s[b:b + 6, cb, 0:128],
                                         gvXs[b:b + 6, cb, 0:CN], start=False, stop=True)
                    sqx = na.tile([128, CN], F32, name=f"sqx{ic}{cb}", tag=f"sqx{cb}")
                    sqy = na.tile([128, CN], F32, name=f"sqy{ic}{cb}", tag=f"sqy{cb}")
                    gyc = na.tile([128, CN], F32, name=f"gyc{ic}{cb}", tag=f"gyc{cb}")
                    nc.scalar.activation(sqx[:], pg[0][:, 0:CN], ACT.Square)
                    nc.scalar.activation(sqy[:], pg[1][:, 0:CN], ACT.Square)
                    nc.scalar.copy(gyc[:], pg[1][:, 0:CN])
                    nc.vector.tensor_tensor(out=gyc[:].bitcast(U32), in0=pg[0][:, 0:CN].bitcast(U32),
                                            in1=gyc[:].bitcast(U32), op=OP.bitwise_xor)
                    nc.gpsimd.tensor_tensor(out=magb[:, cb, lo:hi], in0=sqx[:], in1=sqy[:], op=OP.add)
                    sqtiles[(ic, cb)] = (sqx, sqy, gyc[:].bitcast(U32))

            def conv_post(ic):
                lo, hi = RCH[ic]
                for cb in range(NCB):
                    sqx, sqy, xr = sqtiles[(ic, cb)]
                    nc.vector.tensor_scalar(out=pmask[:, cb, lo:hi], in0=xr,
                                            scalar1=2147483648, scalar2=None, op0=OP.is_lt)
                    nc.vector.scalar_tensor_tensor(out=maskH[:, cb, lo:hi], in0=sqx[:],
                                                   scalar=t2c, in1=sqy[:],
                                                   op0=OP.mult, op1=OP.is_ge)
                    nc.vector.scalar_tensor_tensor(out=maskV[:, cb, lo:hi], in0=sqy[:],
                                                   scalar=t2c, in1=sqx[:],
                                                   op0=OP.mult, op1=OP.is_gt)

            def shift_dmas(jc):
                nlo, nhi = NCH[jc]
                ra, rb = nlo - 1, nhi + 1
                CNH = rb - ra
                nc.sync.dma_start(out=m2L[jc][1:128, :, 0:CNH], in_=magb[0:127, :, ra:rb])
                nc.sync.dma_start(out=m2L[jc][0:1, 1:8, 0:CNH], in_=magb[127:128, 0:7, ra:rb])
                nc.scalar.dma_start(out=m2R[jc][0:127, :, 0:CNH], in_=magb[1:128, :, ra:rb])
                nc.scalar.dma_start(out=m2R[jc][127:128, 0:7, 0:CNH], in_=magb[0:1, 1:8, ra:rb])

            def nms_chunk(jc):
                nlo, nhi = NCH[jc]
                CN = nhi - nlo
                ra = nlo - 1
                c0 = slice(nlo, nhi)
                cm = slice(nlo - 1, nhi - 1)
                cp_ = slice(nlo + 1, nhi + 1)
                lc0 = slice(1, 1 + CN)
                lcm = slice(0, CN)
                lcp = slice(2, 2 + CN)
                L, R = m2L[jc], m2R[jc]
                nbsel_t = nb.tile([128, NCB, CN], F32, name=f"nbsel{jc}", tag="nbsel")
                tmp_t = nb.tile([128, NCB, CN], F32, name=f"tmp{jc}", tag="tmp")
                nbsel, tmp = nbsel_t[:], tmp_t[:]
                msk = nbsel
                nc.vector.tensor_tensor(out=nbsel, in0=L[:, :, lcp], in1=R[:, :, lcm], op=OP.max)
                nc.vector.tensor_tensor(out=tmp, in0=R[:, :, lcp], in1=L[:, :, lcm], op=OP.max)
                nc.vector.copy_predicated(nbsel, pmask[:, :, c0], tmp)
                nc.vector.tensor_tensor(out=tmp, in0=magb[:, :, cm], in1=magb[:, :, cp_], op=OP.max)
                nc.vector.copy_predicated(nbsel, maskV[:, :, c0], tmp)
                nc.vector.tensor_tensor(out=tmp, in0=L[:, :, lc0], in1=R[:, :, lc0], op=OP.max)
                nc.vector.copy_predicated(nbsel, maskH[:, :, c0], tmp)
                nc.vector.tensor_tensor(out=tmp, in0=magb[:, :, c0], in1=nbsel, op=OP.is_gt)
                nc.vector.scalar_tensor_tensor(out=msk, in0=magb[:, :, c0], scalar=1e-6,
                                               in1=tmp, op0=OP.add, op1=OP.mult)
                nc.vector.tensor_scalar(out=smb[:, :, c0], in0=msk,
                                        scalar1=SM2, scalar2=None, op0=OP.is_gt)
                nc.vector.tensor_scalar(out=wmb[:, :, c0], in0=msk,
                                        scalar1=WM2, scalar2=None, op0=OP.is_gt)
                nc.scalar.activation(magob[:, :, c0], msk, ACT.Sqrt)

            pk = make_pack(nb)
            halo_fill(0)
            conv_mm(0)
            with tc.tile_pool(name="pvb", bufs=1, space="PSUM") as pvb, \
                 tc.tile_pool(name="ptb", bufs=2, space="PSUM") as ptb:
                vconv_pass([3, 4], pvb, ptb)
            conv_post(0)
            shift_dmas(0)
            nms_chunk(0)
            pack_part(pk, nc.vector, smb, Sw, *PKW[0])
            pack_part(pk, nc.vector, wmb, Ww, *PKW[0])
            for jc in (1, 2):
                halo_fill(jc)
                conv_mm(jc)
                conv_post(jc)
                shift_dmas(jc)
                nms_chunk(jc)
                if jc == 2:
                    flood_iter(slice(1, 11))   # early iter 0 on packed words; stale-zero seam
                pack_part(pk, nc.vector, smb, Sw, *PKW[jc])
                pack_part(pk, nc.vector, wmb, Ww, *PKW[jc])
        es_x.close()
    es_mask.close()
    if STAGE < 4:
        return
    # ---- scope 4: flood fill, unpack, transpose-out, DMA out ----
    if True:
        es_out = ExitStack()
        stp = es_out.enter_context(tc.tile_pool(name="st", bufs=1))
        pto = es_out.enter_context(tc.tile_pool(name="pto", bufs=4, space="PSUM"))
        def emit_output(oi, src_t, dst, r0off, bf=False):
            stage = stp.tile([128, NT, 1024], F32, name=f"stage{oi}", tag="stage")
            for T in range(NT):
                BT = BTS[T]
                for g in range(2):
                    ptile = pto.tile([128, 512], BF16 if bf else F32, name=f"pto{oi}{T}{g}", tag="pto")
                    for k in range(4):
                        cb = 4 * g + k
                        if bf:
                            nc.tensor.transpose(
                                ptile[0:BT, 128 * k:128 * k + 128],
                                src_t[:, cb, r0off + 128 * T: r0off + 128 * T + BT],
                                identb[:])
                        else:
                            nc.tensor.transpose(
                                r32(ptile[0:BT, 128 * k:128 * k + 128]),
                                r32(src_t[:, cb, r0off + 128 * T: r0off + 128 * T + BT]),
                                ident[:])
                    nc.scalar.copy(stage[0:BT, T, 512 * g:512 * g + 512], ptile[0:BT, :])
            for T in range(4):
                nc.sync.dma_start(
                    out=_ap(dst, 128 * T * 1024, [[1024, 128], [1, 1024]]),
                    in_=stage[:, T, :])
            nc.sync.dma_start(
                out=_ap(dst, 512 * 1024, [[1024, 17], [1, 1024]]),
                in_=stage[0:17, 4, :])
        emit_output(0, magob, mag_out, 1, bf=True)
        for it in range(1, ITERS):
            flood_iter(slice(1, 18))

        if STAGE < 6:
            return
        edgesT = stp.tile([128, NCB, PACK_ROWS], F32R, tag="edgesT")
        eti = stp.tile([128, NCB, PACK_ROWS], U32, tag="eti")
        stage = stp.tile([128, NT, 1024], F32, name="stageE", tag="stage")
        pap = pat_s[:, :]
        for T in range(NT):
            BT = BTS[T]
            W = 4 if T < 4 else 1
            sap = Sw[:, :, 1 + 4 * T:1 + 4 * T + W]
            bits_in = bass.AP(sap.tensor, sap.offset, list(sap.ap) + [[0, 32]])
            pat_bc = bass.AP(pap.tensor, pap.offset, [list(pap.ap[0]), [0, NCB], [0, W], list(pap.ap[1])])
            rsl = slice(128 * T, 128 * T + 32 * W)
            nc.vector.tensor_tensor(out=eti[:, :, rsl].rearrange("p c (j k) -> p c j k", k=32),
                                    in0=bits_in, in1=pat_bc, op=OP.bitwise_and)
            nc.vector.tensor_scalar(out=edgesT[:, :, rsl], in0=eti[:, :, rsl],
                                    scalar1=0, scalar2=None, op0=OP.not_equal)
            for g in range(2):
                ptile = pto.tile([128, 512], F32, name=f"ptoE{T}{g}", tag="pto")
                for k in range(4):
                    cb = 4 * g + k
                    nc.tensor.transpose(
                        r32(ptile[0:BT, 128 * k:128 * k + 128]),
                        r32(edgesT[:, cb, 128 * T:128 * T + BT]),
                        ident[:])
                nc.scalar.copy(stage[0:BT, T, 512 * g:512 * g + 512], ptile[0:BT, :])
            nc.sync.dma_start(
                out=_ap(edges_out, 128 * T * 1024, [[1024, BT], [1, 1024]]),
                in_=stage[0:BT, T, :])
        es_out.close()


_CACHE = {}

def _build():
    if 'nc' in _CACHE:
        return
    import concourse.bacc as bacc
    import concourse.mybir as mybir
    import concourse.tile as tile
    from contextlib import ExitStack
    patT, wvT, whT, whxT, metaT = make_core_inputs(True)
    patB, wvB, whB, whxB, metaB = make_core_inputs(False)
    assert metaT == metaB
    nc = bacc.Bacc("TRN2", target_bir_lowering=False, debug=False)
    x = nc.dram_tensor("x", [3, XP, 1024], mybir.dt.float32, kind="ExternalInput")
    wv = nc.dram_tensor("wv", list(wvT.shape), mybir.dt.float32, kind="ExternalInput")
    wh = nc.dram_tensor("wh", list(whT.shape), mybir.dt.float32, kind="ExternalInput")
    whx = nc.dram_tensor("whx", list(whxT.shape), mybir.dt.float32, kind="ExternalInput")
    pat = nc.dram_tensor("pat", [128, 32], mybir.dt.uint32, kind="ExternalInput")
    identt = nc.dram_tensor("ident", [128, 128], mybir.dt.float32r, kind="ExternalInput")
    mag_o = nc.dram_tensor("mag_o", [529, 1024], mybir.dt.float32, kind="ExternalOutput")
    edges_o = nc.dram_tensor("edges_o", [529, 1024], mybir.dt.float32, kind="ExternalOutput")
    with ExitStack() as ctx:
        tc = ctx.enter_context(tile.TileContext(nc))
        canny_core(ctx, tc, [mag_o.ap(), edges_o.ap()],
                   [x.ap(), wv.ap(), wh.ap(), whx.ap(), pat.ap(), identt.ap()], metaT)
    nc.finalize()
    _CACHE.update(nc=nc, weights=dict(top=(patT, wvT, whT, whxT), bot=(patB, wvB, whB, whxB)))

def kernel(x):
    _build()
    from concourse.bass_utils import run_bass_kernel_spmd
    nc = _CACHE['nc']
    x = np.ascontiguousarray(np.asarray(x, dtype=np.float32))
    B = x.shape[0]
    in_maps = []
    for core in range(8):
        b, half = core // 2, core % 2
        top = (half == 0)
        pad = np.zeros((3, 3, 1024), np.float32)
        if top:
            xw = np.concatenate([pad, x[b, :, 0:532, :]], axis=1)
        else:
            xw = np.concatenate([x[b, :, 492:1024, :], pad], axis=1)
        patc, wvc, whc, whxc = _CACHE['weights']['top' if top else 'bot']
        in_maps.append({"x": np.ascontiguousarray(xw), "wv": wvc, "wh": whc, "whx": whxc,
                        "pat": patc, "ident": np.eye(128, dtype=np.float32)})
    res = run_bass_kernel_spmd(nc, in_maps, core_ids=list(range(8)))
    mag = np.zeros((B, 1, 1024, 1024), np.float32)
    edges = np.zeros((B, 1, 1024, 1024), np.float32)
    for core in range(8):
        b, half = core // 2, core % 2
        r = res.results[core]
        if half == 0:
            mag[b, 0, 0:512] = r["mag_o"][0:512]
            edges[b, 0, 0:512] = r["edges_o"][0:512]
        else:
            mag[b, 0, 512:1024] = r["mag_o"][17:529]
            edges[b, 0, 512:1024] = r["edges_o"][17:529]
    return mag, edges



# revision 48
# speedup vs baseline: 1.0212x; 1.0010x over previous
"""Self-contained Trainium2 Bass kernel for Canny edge detection (4,3,1024,1024).

kernel(x) -> (magnitude, edges), each [4,1,1024,1024] f32. 8 NeuronCores SPMD:
core = (batch, image half); no cross-core communication (flood-fill halo margin).
"""
import numpy as np

XR = 532          # x window rows per core
NM = 529          # mag rows per core
H_IMG, W_IMG = 1024, 1024
RD = 532          # r-dim of mag-grid col-major buffers: slot = 1+M, guards at 0,530
WSLOT = 19        # flood word slots: 0 guard, 1..17 data, 18 guard
PACK_ROWS = 544   # 17 words * 32 rows
ITERS = 4
GRAY_W = np.array([0.299, 0.587, 0.114], np.float32)
TAN225 = np.float32(np.tan(np.pi / 8))  # 0.41421356

def thresh2(c):
    """Largest f32 v* with (v > v*) == (f32(sqrt(v)) > c) for f32 v; NMS thresholds on m2+eps."""
    c = np.float32(c)
    v = np.float32(c * c)
    while np.float32(np.sqrt(v)) > c:
        v = np.nextafter(v, np.float32(0), dtype=np.float32)
    while np.float32(np.sqrt(np.nextafter(v, np.float32(np.inf), dtype=np.float32))) <= c:
        v = np.nextafter(v, np.float32(np.inf), dtype=np.float32)
    return float(v)

def gauss5():
    # f32 replica of reference._gaussian_kernel1d(5, 1.0)
    x = (np.arange(5, dtype=np.float32) - 2).astype(np.float32)
    g = np.exp((-x * x / np.float32(2.0)).astype(np.float32)).astype(np.float32)
    return (g / g.sum(dtype=np.float32)).astype(np.float32)

def _op_conv(n_out, n_in, taps, center, idx_map):
    """Row t of output = sum_d taps[d] * in[idx_map(t + d - center)], f64 build."""
    C = np.zeros((n_out, n_in), np.float64)
    for t in range(n_out):
        for d, w in enumerate(taps):
            s = idx_map(t + d - center)
            C[t, s] += w
    return C

def reflect_idx(i, n):
    # jnp.pad 'reflect': -1 -> 1, -2 -> 2; n -> n-2, n+1 -> n-3
    if i < 0:
        return -i
    if i >= n:
        return 2 * n - 2 - i
    return i

def clamp_idx(i, n):
    return min(max(i, 0), n - 1)

def build_vertical_ops(top: bool):
    """Return (Cvx, Cvy): [NM, XR] f32 composed vertical operators for this core."""
    g = gauss5().astype(np.float64)
    # Stage 1: gauss vertical with reflect at IMAGE edges, over x window rows.
    # blurred-v needed rows: image rows of M-1 .. M+1 -> local B = -1..529
    # local->img: top: img = local_x;  bottom: img = 492 + local_x
    # blurV local grid b = -1..529 maps to img rows (top: b, bottom: 492+b... wait
    #   bottom mag M -> img 495+M; blur rows needed img 494..1024)
    # Build on local-x axis directly with the correct edge behavior:
    #   top: local 0 == img 0 (reflect boundary at local 0); far end interior.
    #   bottom: local 531 == img 1023 (reflect boundary there); near end interior.
    NB = 531  # blur rows b = -1..529 stored t = b+1
    def xmap_top(i):   # reflect at 0 only (other end never reached out of range)
        return reflect_idx(i, 10**9) if i >= 0 else -i
    def xmap_bot(i):
        if i >= XR:
            return 2 * XR - 2 - i
        return i
    xmap = xmap_top if top else xmap_bot
    # blur b (local-x coordinate of output): top: b = t-1; bottom: b = t-1+2
    #   top: blurV[b] centered at x local row b;  b from -1..529
    #   bottom: mag M -> img 495+M -> local x = 495+M-492 = 3+M; blur rows local x = 2+M-1.. 
    #     blur grid b(local x) = 2 .. 532 for t=0..530
    off = 0 if top else 3
    Cb = np.zeros((NB, XR), np.float64)
    for t in range(NB):
        b = t - 1 + off   # local-x row this blurV output is centered on
        for d in range(5):
            s = b + d - 2
            s = xmap(s)
            assert 0 <= s < XR, (top, t, s)
            Cb[t, s] += g[d]
    # Stage 2: sobel vertical ops on the blurV grid with replicate at IMAGE edges.
    # mag M: taps at blur rows b = (M-1 .. M+1) in local-x => stored t = M-1+1..=M..M+2 - wait
    #   stored t of blur row b: t = b + 1 - off ... b_local_x = M + off + db where db=-1..1
    #   stored t = (M + off + db) - off + 1 - ... let me just: stored t corresponds to b_lx = t-1+off
    #   For mag M: need b_lx = (M+off) + db  => t = M + 1 + db
    # replicate at image edges: top: b_lx < 0 -> 0 i.e. t<0 -> t=0? replicate on blur IMG rows:
    #   top: blur img row = b_lx; replicate row<0 -> row 0 -> t index of b_lx=0 is t=1.
    #   bottom: blur img row = 492 + b_lx; replicate row>1023 -> b_lx>531 -> clamp to 531 (t=530)
    vsm = np.array([1.0, 2.0, 1.0])
    vdf = np.array([-1.0, 0.0, 1.0])
    Cvx = np.zeros((NM, XR), np.float64)
    Cvy = np.zeros((NM, XR), np.float64)
    for M in range(NM):
        for db, (wx, wy) in enumerate(zip(vsm, vdf)):
            t = M + db  # t = M+1+(db-1)
            if top:
                t = max(t, 1)       # replicate img row 0 (t=1)... t=0 is b_lx=-1 (img -1)
            else:
                t = min(t, NB - 2)  # replicate img row 1023 at far end (t=529)
            # also clamp other end (never used beyond range by construction)
            t = min(max(t, 0), NB - 1)
            Cvx[M] += wx * Cb[t]
            Cvy[M] += wy * Cb[t]
    return Cvx.astype(np.float32), Cvy.astype(np.float32)

def build_horizontal_ops():
    """(Chx, Chy): [W, W] composed horizontal operators (same both cores)."""
    g = gauss5().astype(np.float64)
    Cb = _op_conv(W_IMG, W_IMG, g, 2, lambda i: reflect_idx(i, W_IMG))
    Dif = _op_conv(W_IMG, W_IMG, [-1.0, 0.0, 1.0], 1, lambda i: clamp_idx(i, W_IMG))
    Sm = _op_conv(W_IMG, W_IMG, [1.0, 2.0, 1.0], 1, lambda i: clamp_idx(i, W_IMG))
    Chx = (Dif @ Cb).astype(np.float32)
    Chy = (Sm @ Cb).astype(np.float32)
    return Chx, Chy

# ---------------- numpy model of the per-core pipeline (for validation) -------------
def core_model(x_win, top):
    """x_win: [3, XR, 1024] f32. Returns (magout [NM,1024], edges [NM,1024])."""
    Cvx, Cvy = build_vertical_ops(top)
    Chx, Chy = build_horizontal_ops()
    gray = np.tensordot(GRAY_W, x_win.astype(np.float32), 1)  # [XR, W]
    gvx = (Cvx @ gray).astype(np.float32)
    gvy = (Cvy @ gray).astype(np.float32)
    gx = (gvx @ Chx.T).astype(np.float32)
    gy = (gvy @ Chy.T).astype(np.float32)
    m2 = gx * gx + gy * gy
    mag = np.sqrt(m2 + np.float32(1e-6)).astype(np.float32)
    magp = np.zeros((NM + 2, W_IMG + 2), np.float32)
    magp[1:-1, 1:-1] = mag
    ax, ay = np.abs(gx), np.abs(gy)
    maskH = (TAN225 * ax) >= ay
    maskV = (TAN225 * ay) > ax
    pmask = (gx * gy) >= 0
    c = magp[1:-1, 1:-1]
    up, dn = magp[0:-2, 1:-1], magp[2:, 1:-1]
    lf, rt = magp[1:-1, 0:-2], magp[1:-1, 2:]
    ul, ur = magp[0:-2, 0:-2], magp[0:-2, 2:]
    dl, dr = magp[2:, 0:-2], magp[2:, 2:]
    nbH = np.maximum(lf, rt); nbV = np.maximum(up, dn)
    nbD1 = np.maximum(dr, ul); nbD2 = np.maximum(dl, ur)
    nbsel = nbD2.copy()
    nbsel[pmask] = nbD1[pmask]
    nbsel[maskV] = nbV[maskV]
    nbsel[maskH] = nbH[maskH]
    ismax = c > nbsel
    magout = mag * ismax
    sm = magout > np.float32(0.2)
    wm = magout > np.float32(0.1)
    S = sm.copy(); W = wm
    for _ in range(ITERS):
        Sp = np.zeros((NM + 2, W_IMG + 2), bool)
        Sp[1:-1, 1:-1] = S
        dil = Sp[0:-2,0:-2]|Sp[0:-2,1:-1]|Sp[0:-2,2:]|Sp[1:-1,0:-2]|Sp[1:-1,1:-1]|Sp[1:-1,2:]|Sp[2:,0:-2]|Sp[2:,1:-1]|Sp[2:,2:]
        S = S | (W & dil)
    return magout, S.astype(np.float32)


import numpy as np
from collections import defaultdict
import concourse.bass as bass
import concourse.mybir as mybir
from concourse.masks import make_identity


F32, I32, U32, U8 = mybir.dt.float32, mybir.dt.int32, mybir.dt.uint32, mybir.dt.uint8
F32R = mybir.dt.float32r
BF16 = mybir.dt.bfloat16
OP = mybir.AluOpType
ACT = mybir.ActivationFunctionType
MASK_DT = U8

NT = 5
BTS = [128, 128, 128, 128, 17]
NCB = 8
RCH = [(1, 162), (162, 354), (354, 530)]     # conv r-slot chunks
NCH4 = [(1, 161), (161, 353), (353, 530)]    # NMS r-slot chunks (word-aligned)
PKW = [(0, 5), (5, 11), (11, 17)]            # pack word ranges per NMS chunk

XP = 535           # padded x rows: top = [0,0,0, img 0..531]; bottom = [img 492..1023, 0,0,0]
VBKS = [122, 122, 122, 122, 41]   # output rows per vertical block (sum 529)
VKS = [128, 128, 128, 128, 47]    # input rows per block, start = 122*k

def build_vplan(top):
    Cvx, Cvy = build_vertical_ops(top)
    w = np.float64(np.float32(GRAY_W[2]))  # 0.114 folded out of the DVE gray stage
    # pad to the unified 535-row local axis
    pads = []
    for C in (Cvx, Cvy):
        Cp = np.zeros((NM, XP), np.float64)
        if top:
            Cp[:, 3:3 + XR] = C
        else:
            Cp[:, 0:XR] = C
        pads.append(Cp)
    arr = np.zeros((128, 2 * NT, 128), np.float32)
    for ci, C in enumerate(pads):
        for k in range(NT):
            r0, BK, K = 122 * k, VBKS[k], VKS[k]
            sub = C[r0:r0 + BK, :]
            assert np.all(sub[:, :122 * k] == 0) and np.all(sub[:, 122 * k + K:] == 0), (top, ci, k)
            arr[0:K, ci * NT + k, 0:BK] = (w * sub[:, 122 * k:122 * k + K]).T
    return arr

def build_hplan():
    """wh [128, 16, 128]: slot ci*8+cb = dense diagonal block (input cols 128cb..+127).
    whx [8, 16, 128]: halo block: rows 0..2 = input cols 128cb-3..-1, rows 3..5 = 128cb+128..+130."""
    Chx, Chy = build_horizontal_ops()
    wh = np.zeros((128, 2 * NCB, 128), np.float32)
    whx = np.zeros((64, NCB, 128), np.float32)
    for ci, C in enumerate((Chx, Chy)):
        for cb in range(NCB):
            p0 = 128 * cb
            s = ci * NCB + cb
            b = 32 * ci
            wh[:, s, :] = C[p0:p0 + 128, p0:p0 + 128].T
            if cb > 0:
                whx[b:b + 3, cb, :] = C[p0:p0 + 128, p0 - 3:p0].T
            if cb < NCB - 1:
                whx[b + 3:b + 6, cb, :] = C[p0:p0 + 128, p0 + 128:p0 + 131].T
            assert np.all(C[p0:p0 + 128, :max(p0 - 3, 0)] == 0)
            assert np.all(C[p0:p0 + 128, p0 + 131:] == 0)
    return wh, whx

def pack_blocks(blocks, kinds):
    """kinds[i] in {'full','lo32','hi32'}; hi32 must land at k0=96, lo32/full at 0."""
    places = [None] * len(blocks)
    slots = []
    free_lo, free_hi = [], []
    for i, (b, kind) in enumerate(zip(blocks, kinds)):
        if kind == 'full':
            slots.append([])
            slots[-1].append((0, b))
            places[i] = (len(slots) - 1, 0)
        elif kind == 'lo32':
            if not free_lo:
                slots.append([])
                free_hi.append(len(slots) - 1)
                free_lo.append(len(slots) - 1)
            s = free_lo.pop(0)
            slots[s].append((0, b))
            places[i] = (s, 0)
        else:  # hi64 at k0=64
            if not free_hi:
                slots.append([])
                free_lo.append(len(slots) - 1)
                free_hi.append(len(slots) - 1)
            s = free_hi.pop(0)
            slots[s].append((64, b))
            places[i] = (s, 64)
    arr = np.zeros((128, len(slots), 128), np.float32)
    for slot, entries in enumerate(slots):
        for k0, b in entries:
            K, M = b.shape
            arr[k0:k0 + K, slot, 0:M] = b
    return arr, places

def make_core_inputs(top):
    wv = build_vplan(top)
    wh, whx = build_hplan()
    pat = np.tile(np.uint32(1) << np.arange(32, dtype=np.uint32), (128, 1))
    meta = dict(nv=wv.shape[1], nh=wh.shape[1])
    return np.ascontiguousarray(pat), wv, wh, whx, meta

def _ap(base_ap, offset_elems, dims):
    return bass.AP(base_ap.tensor, base_ap.offset + offset_elems, dims)

def r32(ap):
    return ap.bitcast(mybir.dt.float32r)

def stt_u32(nc, out, in0, scalar, in1, op0, op1):
    """scalar_tensor_tensor with an integer (u32) immediate, for bitvec ops."""
    eng = nc.vector
    return eng.add_instruction(
        mybir.InstTensorScalarPtr(
            name=nc.get_next_instruction_name(),
            is_scalar_tensor_tensor=True,
            op0=op0,
            op1=op1,
            ins=[eng.lower_ap(in0),
                 mybir.ImmediateValue(dtype=mybir.dt.uint32, value=scalar),
                 eng.lower_ap(in1)],
            outs=[eng.lower_ap(out)],
        ))

def canny_core(ctx, tc, outs, ins, meta):
    import os
    STAGE = int(os.environ.get('CANNY_STAGE', '9'))
    from contextlib import ExitStack
    nc = tc.nc
    mag_out, edges_out = outs
    x_in, wv_in, wh_in, whx_in, pat_in, ident_in = ins
    NVS, NHS = meta['nv'], meta['nh']

    consts = ctx.enter_context(tc.tile_pool(name="consts", bufs=1))
    pat_s = consts.tile([128, 32], U32)
    nc.sync.dma_start(pat_s[:], pat_in)
    ident = consts.tile([128, 128], F32R)
    nc.sync.dma_start(ident[:], ident_in)
    identb = consts.tile([128, 128], BF16)
    make_identity(nc, identb)

    persist = ctx.enter_context(tc.tile_pool(name="persist", bufs=1))
    magb = persist.tile([128, NCB, RD], F32)          # holds m2 = gx^2+gy^2
    magob = persist.tile([128, NCB, RD], BF16)        # final masked magnitude (bf16 ok: post-decision values)
    nc.gpsimd.memset(magb[:, :, 0:1], 0.0)
    nc.gpsimd.memset(magb[:, :, 530:532], 0.0)
    m2L_s = persist.tile([128, NCB, 194], F32, name="m2L")
    m2R_s = persist.tile([128, NCB, 194], F32, name="m2R")
    m2L = [m2L_s, m2L_s, m2L_s]
    m2R = [m2R_s, m2R_s, m2R_s]
    nc.gpsimd.memset(m2L_s[0:1, 0:1, :], 0.0)
    nc.gpsimd.memset(m2R_s[96:128, 7:8, :], 0.0)

    swp = ctx.enter_context(tc.tile_pool(name="swp", bufs=1))
    smb = swp.tile([128, NCB, PACK_ROWS + 2], U8)
    wmb = swp.tile([128, NCB, PACK_ROWS + 2], U8)
    nc.gpsimd.memset(smb[:, :, 530:546], 0)
    nc.gpsimd.memset(wmb[:, :, 530:546], 0)
    fl = ctx.enter_context(tc.tile_pool(name="fl", bufs=1))
    Sw = fl.tile([128, NCB, WSLOT], U32, tag="Sw")
    Ww = fl.tile([128, NCB, WSLOT], U32, tag="Ww")
    HL = fl.tile([128, NCB, WSLOT], U32, tag="HL")
    HR = fl.tile([128, NCB, WSLOT], U32, tag="HR")
    Hd = fl.tile([128, NCB, WSLOT], U32, tag="Hd")
    Vd = fl.tile([128, NCB, WSLOT], U32, tag="Vd")
    ta = fl.tile([128, NCB, WSLOT], U32, tag="ta")
    for t in (Sw, Ww, HL, HR, Hd, Vd, ta):
        nc.gpsimd.memset(t[:], 0)
    def flood_iter(dw):
        a, b = dw.start, dw.stop
        dm, dp = slice(a - 1, b - 1), slice(a + 1, b + 1)
        nc.sync.dma_start(out=HL[1:128, :, dw], in_=Sw[0:127, :, dw])
        nc.scalar.dma_start(out=HL[0:1, 1:8, dw], in_=Sw[127:128, 0:7, dw])
        nc.gpsimd.dma_start(out=HR[0:127, :, dw], in_=Sw[1:128, :, dw])
        nc.scalar.dma_start(out=HR[127:128, 0:7, dw], in_=Sw[0:1, 1:8, dw])
        nc.vector.tensor_tensor(out=Hd[:, :, dw], in0=Sw[:, :, dw], in1=HL[:, :, dw], op=OP.bitwise_or)
        nc.vector.tensor_tensor(out=Hd[:, :, dw], in0=Hd[:, :, dw], in1=HR[:, :, dw], op=OP.bitwise_or)
        stt_u32(nc, Vd[:, :, dw], Hd[:, :, dw], 1,
                Hd[:, :, dw], OP.logical_shift_left, OP.bitwise_or)
        stt_u32(nc, Vd[:, :, dw], Hd[:, :, dm], 31,
                Vd[:, :, dw], OP.logical_shift_right, OP.bitwise_or)
        stt_u32(nc, Vd[:, :, dw], Hd[:, :, dw], 1,
                Vd[:, :, dw], OP.logical_shift_right, OP.bitwise_or)
        stt_u32(nc, Vd[:, :, dw], Hd[:, :, dp], 31,
                Vd[:, :, dw], OP.logical_shift_left, OP.bitwise_or)
        nc.vector.tensor_tensor(out=ta[:, :, dw], in0=Ww[:, :, dw], in1=Vd[:, :, dw], op=OP.bitwise_and)
        nc.vector.tensor_tensor(out=Sw[:, :, dw], in0=Sw[:, :, dw], in1=ta[:, :, dw], op=OP.bitwise_or)

    def make_pack(pool):
        pk_l1 = pool.tile([128, NCB, 272], BF16, name="l1", tag="l1")
        pk_l2 = pool.tile([128, NCB, 136], BF16, name="l2", tag="l2")
        pk_l3 = pool.tile([128, NCB, 68], BF16, name="l3", tag="l3")
        pk_li = pool.tile([128, NCB, 34], U32, name="li", tag="li")
        pk_lsh = pool.tile([128, NCB, 17], U32, name="lsh", tag="lsh")
        return pk_l1, pk_l2, pk_l3, pk_li, pk_lsh

    def pack_part(pk, eng, srcf, dstw, w0, w1):
        l1, l2, l3, li, lsh = pk
        s_hi = srcf[:, :, 2 + 32 * w0:2 + 32 * w1:2]
        s_lo = srcf[:, :, 1 + 32 * w0:1 + 32 * w1:2]
        l1w = l1[:, :, 16 * w0:16 * w1]
        l1r = (l1[:, :, 16 * w0 + 1:16 * w1:2], l1[:, :, 16 * w0:16 * w1 - 1:2])
        l2w = l2[:, :, 8 * w0:8 * w1]
        l2r = (l2[:, :, 8 * w0 + 1:8 * w1:2], l2[:, :, 8 * w0:8 * w1 - 1:2])
        l3w = l3[:, :, 4 * w0:4 * w1]
        l3r = (l3[:, :, 4 * w0 + 1:4 * w1:2], l3[:, :, 4 * w0:4 * w1 - 1:2])
        liw = li[:, :, 2 * w0:2 * w1]
        lshw = lsh[:, :, w0:w1]
        lshr = (li[:, :, 2 * w0 + 1:2 * w1:2], li[:, :, 2 * w0:2 * w1 - 1:2])
        dw = dstw[:, :, 1 + w0:1 + w1]
        eng.scalar_tensor_tensor(out=l1w, in0=s_hi, scalar=2.0, in1=s_lo, op0=OP.mult, op1=OP.add)
        eng.scalar_tensor_tensor(out=l2w, in0=l1r[0], scalar=4.0, in1=l1r[1], op0=OP.mult, op1=OP.add)
        eng.scalar_tensor_tensor(out=l3w, in0=l2r[0], scalar=16.0, in1=l2r[1], op0=OP.mult, op1=OP.add)
        eng.scalar_tensor_tensor(out=liw, in0=l3r[0], scalar=256.0, in1=l3r[1], op0=OP.mult, op1=OP.add)
        nc.vector.tensor_scalar(out=lshw, in0=lshr[0], scalar1=16, scalar2=None, op0=OP.logical_shift_left)
        nc.vector.tensor_tensor(out=dw, in0=lshr[1], in1=lshw, op=OP.bitwise_or)
    es_mask = ctx.enter_context(ExitStack())
    maskp = es_mask.enter_context(tc.tile_pool(name="maskp", bufs=1))
    maskH = maskp.tile([128, NCB, RD], MASK_DT, name="maskH")
    maskV = maskp.tile([128, NCB, RD], MASK_DT, name="maskV")
    pmask = maskp.tile([128, NCB, RD], MASK_DT, name="pmask")

    with tc.tile_pool(name="gvt", bufs=1) as gvtp:
        gvT = [gvtp.tile([128, NCB, RD], F32, name=f"gvT{i}", tag=f"gvT{i}") for i in range(2)]
        wh_s = gvtp.tile([128, NHS, 128], F32)
        nc.sync.dma_start(wh_s[:], wh_in)
        whx_s = gvtp.tile([64, NCB, 128], F32)
        nc.sync.dma_start(whx_s[:], whx_in)
        gvXs = gvtp.tile([64, NCB, 194], F32, name="gvXs")
        nc.gpsimd.memset(gvXs[:], 0.0)
        # ---- vertical convs in two passes, H chunk 0 interleaved between ----
        es_x = ExitStack()
        xp = es_x.enter_context(tc.tile_pool(name="xp", bufs=1))
        gr = es_x.enter_context(tc.tile_pool(name="gr", bufs=2))
        wv_s = xp.tile([128, NVS, 128], F32)
        nc.sync.dma_start(wv_s[:], wv_in)
        xtiles = {}

        def load_chunk(S):
            nrows = VKS[S]
            g = xp.tile([128, 1024], F32, name=f"gray{S}", tag=f"gray{S % 2}")
            for h in range(2):
                cs = slice(512 * h, 512 * h + 512)
                t = xp.tile([128, 3, 512], F32, name=f"xs{S}{h}", tag=f"xs{h}")
                nc.sync.dma_start(
                    out=t[0:nrows, :, :],
                    in_=_ap(x_in, 122 * S * 1024 + 512 * h,
                            [[1024, nrows], [XP * 1024, 3], [1, 512]]))
                nc.vector.scalar_tensor_tensor(out=g[0:nrows, cs], in0=t[0:nrows, 0, :],
                                               scalar=float(np.float32(0.299) / np.float32(0.587)),
                                               in1=t[0:nrows, 1, :], op0=OP.mult, op1=OP.add)
                nc.vector.scalar_tensor_tensor(out=g[0:nrows, cs], in0=g[0:nrows, cs],
                                               scalar=float(np.float32(0.587) / np.float32(0.114)),
                                               in1=t[0:nrows, 2, :], op0=OP.mult, op1=OP.add)
            xtiles[S] = g

        def vconv_pass(Ts, pvp, ptp):
            for T in Ts:
                load_chunk(T)
                BT, K = VBKS[T], VKS[T]
                for ci in range(2):
                    ps = pvp.tile([128, 1024], F32, name=f"pv{T}{ci}", tag="pv")
                    for nh in range(2):
                        cols = slice(512 * nh, 512 * nh + 512)
                        nc.tensor.matmul(ps[0:BT, cols], wv_s[0:K, ci * NT + T, 0:BT],
                                         xtiles[T][0:K, cols], start=True, stop=True)
                    grm = gr.tile([128, 1024], F32, name=f"grm{T}{ci}", tag="grm")
                    if ci == 0:
                        nc.scalar.copy(grm[0:BT, :], ps[0:BT, :])
                    else:
                        nc.vector.tensor_copy(grm[0:BT, :], ps[0:BT, :])
                    for g in range(2):
                        ptile = ptp.tile([128, 512], F32, name=f"pt{ci}{T}{g}", tag="pt")
                        for k in range(4):
                            cb = 4 * g + k
                            nc.tensor.transpose(
                                ptile[0:128, 128 * k:128 * k + BT],
                                grm[0:BT, 128 * cb:128 * cb + 128],
                                ident[0:BT, 0:BT].bitcast(F32))
                        dst_ap = gvT[ci][:, 4 * g:4 * g + 4, 1 + 122 * T:1 + 122 * T + BT]
                        src_ap = ptile[:].rearrange("p (c b) -> p c b", c=4)[:, :, 0:BT]
                        if ci == 0:
                            nc.scalar.copy(dst_ap, src_ap)
                        else:
                            nc.vector.tensor_copy(dst_ap, src_ap)

        with tc.tile_pool(name="pva", bufs=2, space="PSUM") as pva, \
             tc.tile_pool(name="pta", bufs=3, space="PSUM") as pta:
            vconv_pass([0, 1, 2], pva, pta)
        if STAGE < 2:
            return
        # ---- scope 3: horizontal convs + masks + m2, per-chunk NMS overlapped ----
        NCH = NCH4
        t2c = float(np.float32(TAN225) * np.float32(TAN225))
        SM2, WM2 = thresh2(0.2), thresh2(0.1)
        with tc.tile_pool(name="nmsa", bufs=1) as na, \
             tc.tile_pool(name="nmsb", bufs=1) as nb, \
             tc.tile_pool(name="ph", bufs=2, space="PSUM") as ph:

            sqtiles = {}

            def halo_fill(ic):
                lo, hi = RCH[ic]
                CN = hi - lo
                for ci in range(2):
                    b = 32 * ci
                    # rows b+0:3 <- input cols 128cb-3..-1; rows b+3:6 <- cols 128cb+128..+130
                    nc.sync.dma_start(out=gvXs[b:b + 3, 1:8, 0:CN],
                                      in_=gvT[ci][125:128, 0:7, lo:hi])
                    nc.scalar.dma_start(out=gvXs[b + 3:b + 6, 0:7, 0:CN],
                                        in_=gvT[ci][0:3, 1:8, lo:hi])

            def conv_mm(ic, ph2=None):
                lo, hi = RCH[ic]
                CN = hi - lo
                for cb in range(NCB):
                    pool = ph if (ph2 is None or cb % 2 == 0) else ph2
                    pg = [pool.tile([128, CN], F32, name=f"pg{i}", tag=f"pg{i}") for i in range(2)]
                    for ci in range(2):
                        s = ci * NCB + cb
                        nc.tensor.matmul(pg[ci][:, 0:CN], wh_s[0:128, s, 0:128],
                                         gvT[ci][0:128, cb, lo:hi], start=True, stop=False)
                        b = 32 * ci
                        nc.tensor.matmul(pg[ci][:, 0:CN], whx_s[b:b + 6, cb, 0:128],
                                         gvXs[b:b + 6, cb, 0:CN], start=False, stop=True)
                    sqx = na.tile([128, CN], F32, name=f"sqx{ic}{cb}", tag=f"sqx{cb}")
                    sqy = na.tile([128, CN], F32, name=f"sqy{ic}{cb}", tag=f"sqy{cb}")
                    gyc = na.tile([128, CN], F32, name=f"gyc{ic}{cb}", tag=f"gyc{cb}")
                    nc.scalar.activation(sqx[:], pg[0][:, 0:CN], ACT.Square)
                    nc.scalar.activation(sqy[:], pg[1][:, 0:CN], ACT.Square)
                    nc.scalar.copy(gyc[:], pg[1][:, 0:CN])
                    nc.vector.tensor_tensor(out=gyc[:].bitcast(U32), in0=pg[0][:, 0:CN].bitcast(U32),
                                            in1=gyc[:].bitcast(U32), op=OP.bitwise_xor)
                    nc.gpsimd.tensor_tensor(out=magb[:, cb, lo:hi], in0=sqx[:], in1=sqy[:], op=OP.add)
                    sqtiles[(ic, cb)] = (sqx, sqy, gyc[:].bitcast(U32))

            def conv_post(ic):
                lo, hi = RCH[ic]
                for cb in range(NCB):
                    sqx, sqy, xr = sqtiles[(ic, cb)]
                    nc.vector.tensor_scalar(out=pmask[:, cb, lo:hi], in0=xr,
                                            scalar1=2147483648, scalar2=None, op0=OP.is_lt)
                    nc.vector.scalar_tensor_tensor(out=maskH[:, cb, lo:hi], in0=sqx[:],
                                                   scalar=t2c, in1=sqy[:],
                                                   op0=OP.mult, op1=OP.is_ge)
                    nc.vector.scalar_tensor_tensor(out=maskV[:, cb, lo:hi], in0=sqy[:],
                                                   scalar=t2c, in1=sqx[:],
                                                   op0=OP.mult, op1=OP.is_gt)

            def shift_dmas(jc):
                nlo, nhi = NCH[jc]
                ra, rb = nlo - 1, nhi + 1
                CNH = rb - ra
                nc.sync.dma_start(out=m2L[jc][1:128, :, 0:CNH], in_=magb[0:127, :, ra:rb])
                nc.sync.dma_start(out=m2L[jc][0:1, 1:8, 0:CNH], in_=magb[127:128, 0:7, ra:rb])
                nc.scalar.dma_start(out=m2R[jc][0:127, :, 0:CNH], in_=magb[1:128, :, ra:rb])
                nc.scalar.dma_start(out=m2R[jc][127:128, 0:7, 0:CNH], in_=magb[0:1, 1:8, ra:rb])

            def nms_chunk(jc):
                nlo, nhi = NCH[jc]
                CN = nhi - nlo
                ra = nlo - 1
                c0 = slice(nlo, nhi)
                cm = slice(nlo - 1, nhi - 1)
                cp_ = slice(nlo + 1, nhi + 1)
                lc0 = slice(1, 1 + CN)
                lcm = slice(0, CN)
                lcp = slice(2, 2 + CN)
                L, R = m2L[jc], m2R[jc]
                nbsel_t = nb.tile([128, NCB, CN], F32, name=f"nbsel{jc}", tag="nbsel")
                tmp_t = nb.tile([128, NCB, CN], F32, name=f"tmp{jc}", tag="tmp")
                nbsel, tmp = nbsel_t[:], tmp_t[:]
                msk = nbsel
                nc.vector.tensor_tensor(out=nbsel, in0=L[:, :, lcp], in1=R[:, :, lcm], op=OP.max)
                nc.vector.tensor_tensor(out=tmp, in0=R[:, :, lcp], in1=L[:, :, lcm], op=OP.max)
                nc.vector.copy_predicated(nbsel, pmask[:, :, c0], tmp)
                nc.vector.tensor_tensor(out=tmp, in0=magb[:, :, cm], in1=magb[:, :, cp_], op=OP.max)
                nc.vector.copy_predicated(nbsel, maskV[:, :, c0], tmp)
                nc.vector.tensor_tensor(out=tmp, in0=L[:, :, lc0], in1=R[:, :, lc0], op=OP.max)
                nc.vector.copy_predicated(nbsel, maskH[:, :, c0], tmp)
                nc.vector.tensor_tensor(out=tmp, in0=magb[:, :, c0], in1=nbsel, op=OP.is_gt)
                nc.vector.scalar_tensor_tensor(out=msk, in0=magb[:, :, c0], scalar=1e-6,
                                               in1=tmp, op0=OP.add, op1=OP.mult)
                nc.vector.tensor_scalar(out=smb[:, :, c0], in0=msk,
                                        scalar1=SM2, scalar2=None, op0=OP.is_gt)
                nc.vector.tensor_scalar(out=wmb[:, :, c0], in0=msk,
                                        scalar1=WM2, scalar2=None, op0=OP.is_gt)
                nc.scalar.activation(magob[:, :, c0], msk, ACT.Sqrt)

            pk = make_pack(nb)
            halo_fill(0)
            conv_mm(0)
            with tc.tile_pool(name="pvb", bufs=1, space="PSUM") as pvb, \
                 tc.tile_pool(name="ptb", bufs=2, space="PSUM") as ptb:
                vconv_pass([3, 4], pvb, ptb)
            conv_post(0)
            shift_dmas(0)
            nms_chunk(0)
            pack_part(pk, nc.vector, smb, Sw, *PKW[0])
            pack_part(pk, nc.vector, wmb, Ww, *PKW[0])
            with tc.tile_pool(name="ph2", bufs=2, space="PSUM") as ph2:
                for jc in (1, 2):
                    halo_fill(jc)
                    conv_mm(jc, ph2)
                    conv_post(jc)
                    shift_dmas(jc)
                    nms_chunk(jc)
                    if jc == 2:
                        flood_iter(slice(1, 11))   # early iter 0 on packed words
                    pack_part(pk, nc.vector, smb, Sw, *PKW[jc])
                    pack_part(pk, nc.vector, wmb, Ww, *PKW[jc])
        es_x.close()
    es_mask.close()
    if STAGE < 4:
        return
    # ---- scope 4: flood fill, unpack, transpose-out, DMA out ----
    if True:
        es_out = ExitStack()
        stp = es_out.enter_context(tc.tile_pool(name="st", bufs=1))
        pto = es_out.enter_context(tc.tile_pool(name="pto", bufs=4, space="PSUM"))
        def emit_output(oi, src_t, dst, r0off, bf=False):
            stage = stp.tile([128, NT, 1024], F32, name=f"stage{oi}", tag="stage")
            for T in range(NT):
                BT = BTS[T]
                for g in range(2):
                    ptile = pto.tile([128, 512], BF16 if bf else F32, name=f"pto{oi}{T}{g}", tag="pto")
                    for k in range(4):
                        cb = 4 * g + k
                        if bf:
                            nc.tensor.transpose(
                                ptile[0:BT, 128 * k:128 * k + 128],
                                src_t[:, cb, r0off + 128 * T: r0off + 128 * T + BT],
                                identb[:])
                        else:
                            nc.tensor.transpose(
                                r32(ptile[0:BT, 128 * k:128 * k + 128]),
                                r32(src_t[:, cb, r0off + 128 * T: r0off + 128 * T + BT]),
                                ident[:])
                    nc.scalar.copy(stage[0:BT, T, 512 * g:512 * g + 512], ptile[0:BT, :])
            for T in range(4):
                nc.sync.dma_start(
                    out=_ap(dst, 128 * T * 1024, [[1024, 128], [1, 1024]]),
                    in_=stage[:, T, :])
            nc.sync.dma_start(
                out=_ap(dst, 512 * 1024, [[1024, 17], [1, 1024]]),
                in_=stage[0:17, 4, :])
        emit_output(0, magob, mag_out, 1, bf=True)
        for it in range(1, ITERS):
            flood_iter(slice(1, 18))

        if STAGE < 6:
            return
        edgesT = stp.tile([128, NCB, PACK_ROWS], F32R, tag="edgesT")
        eti = stp.tile([128, NCB, PACK_ROWS], U32, tag="eti")
        stage = stp.tile([128, NT, 1024], F32, name="stageE", tag="stage")
        pap = pat_s[:, :]
        for T in range(NT):
            BT = BTS[T]
            W = 4 if T < 4 else 1
            sap = Sw[:, :, 1 + 4 * T:1 + 4 * T + W]
            bits_in = bass.AP(sap.tensor, sap.offset, list(sap.ap) + [[0, 32]])
            pat_bc = bass.AP(pap.tensor, pap.offset, [list(pap.ap[0]), [0, NCB], [0, W], list(pap.ap[1])])
            rsl = slice(128 * T, 128 * T + 32 * W)
            nc.vector.tensor_tensor(out=eti[:, :, rsl].rearrange("p c (j k) -> p c j k", k=32),
                                    in0=bits_in, in1=pat_bc, op=OP.bitwise_and)
            nc.vector.tensor_scalar(out=edgesT[:, :, rsl], in0=eti[:, :, rsl],
                                    scalar1=0, scalar2=None, op0=OP.not_equal)
            for g in range(2):
                ptile = pto.tile([128, 512], F32, name=f"ptoE{T}{g}", tag="pto")
                for k in range(4):
                    cb = 4 * g + k
                    nc.tensor.transpose(
                        r32(ptile[0:BT, 128 * k:128 * k + 128]),
                        r32(edgesT[:, cb, 128 * T:128 * T + BT]),
                        ident[:])
                nc.scalar.copy(stage[0:BT, T, 512 * g:512 * g + 512], ptile[0:BT, :])
            nc.sync.dma_start(
                out=_ap(edges_out, 128 * T * 1024, [[1024, BT], [1, 1024]]),
                in_=stage[0:BT, T, :])
        es_out.close()


_CACHE = {}

def _build():
    if 'nc' in _CACHE:
        return
    import concourse.bacc as bacc
    import concourse.mybir as mybir
    import concourse.tile as tile
    from contextlib import ExitStack
    patT, wvT, whT, whxT, metaT = make_core_inputs(True)
    patB, wvB, whB, whxB, metaB = make_core_inputs(False)
    assert metaT == metaB
    nc = bacc.Bacc("TRN2", target_bir_lowering=False, debug=False)
    x = nc.dram_tensor("x", [3, XP, 1024], mybir.dt.float32, kind="ExternalInput")
    wv = nc.dram_tensor("wv", list(wvT.shape), mybir.dt.float32, kind="ExternalInput")
    wh = nc.dram_tensor("wh", list(whT.shape), mybir.dt.float32, kind="ExternalInput")
    whx = nc.dram_tensor("whx", list(whxT.shape), mybir.dt.float32, kind="ExternalInput")
    pat = nc.dram_tensor("pat", [128, 32], mybir.dt.uint32, kind="ExternalInput")
    identt = nc.dram_tensor("ident", [128, 128], mybir.dt.float32r, kind="ExternalInput")
    mag_o = nc.dram_tensor("mag_o", [529, 1024], mybir.dt.float32, kind="ExternalOutput")
    edges_o = nc.dram_tensor("edges_o", [529, 1024], mybir.dt.float32, kind="ExternalOutput")
    with ExitStack() as ctx:
        tc = ctx.enter_context(tile.TileContext(nc))
        canny_core(ctx, tc, [mag_o.ap(), edges_o.ap()],
                   [x.ap(), wv.ap(), wh.ap(), whx.ap(), pat.ap(), identt.ap()], metaT)
    nc.finalize()
    _CACHE.update(nc=nc, weights=dict(top=(patT, wvT, whT, whxT), bot=(patB, wvB, whB, whxB)))

def kernel(x):
    _build()
    from concourse.bass_utils import run_bass_kernel_spmd
    nc = _CACHE['nc']
    x = np.ascontiguousarray(np.asarray(x, dtype=np.float32))
    B = x.shape[0]
    in_maps = []
    for core in range(8):
        b, half = core // 2, core % 2
        top = (half == 0)
        pad = np.zeros((3, 3, 1024), np.float32)
        if top:
            xw = np.concatenate([pad, x[b, :, 0:532, :]], axis=1)
        else:
            xw = np.concatenate([x[b, :, 492:1024, :], pad], axis=1)
        patc, wvc, whc, whxc = _CACHE['weights']['top' if top else 'bot']
        in_maps.append({"x": np.ascontiguousarray(xw), "wv": wvc, "wh": whc, "whx": whxc,
                        "pat": patc, "ident": np.eye(128, dtype=np.float32)})
    res = run_bass_kernel_spmd(nc, in_maps, core_ids=list(range(8)))
    mag = np.zeros((B, 1, 1024, 1024), np.float32)
    edges = np.zeros((B, 1, 1024, 1024), np.float32)
    for core in range(8):
        b, half = core // 2, core % 2
        r = res.results[core]
        if half == 0:
            mag[b, 0, 0:512] = r["mag_o"][0:512]
            edges[b, 0, 0:512] = r["edges_o"][0:512]
        else:
            mag[b, 0, 512:1024] = r["mag_o"][17:529]
            edges[b, 0, 512:1024] = r["edges_o"][17:529]
    return mag, edges



# revision 49
# speedup vs baseline: 1.0465x; 1.0248x over previous
"""Self-contained Trainium2 Bass kernel for Canny edge detection (4,3,1024,1024).

kernel(x) -> (magnitude, edges), each [4,1,1024,1024] f32. 8 NeuronCores SPMD:
core = (batch, image half); no cross-core communication (flood-fill halo margin).
"""
import numpy as np

XR = 532          # x window rows per core
NM = 529          # mag rows per core
H_IMG, W_IMG = 1024, 1024
RD = 532          # r-dim of mag-grid col-major buffers: slot = 1+M, guards at 0,530
WSLOT = 19        # flood word slots: 0 guard, 1..17 data, 18 guard
PACK_ROWS = 544   # 17 words * 32 rows
ITERS = 4
GRAY_W = np.array([0.299, 0.587, 0.114], np.float32)
TAN225 = np.float32(np.tan(np.pi / 8))  # 0.41421356

def thresh2(c):
    """Largest f32 v* with (v > v*) == (f32(sqrt(v)) > c) for f32 v; NMS thresholds on m2+eps."""
    c = np.float32(c)
    v = np.float32(c * c)
    while np.float32(np.sqrt(v)) > c:
        v = np.nextafter(v, np.float32(0), dtype=np.float32)
    while np.float32(np.sqrt(np.nextafter(v, np.float32(np.inf), dtype=np.float32))) <= c:
        v = np.nextafter(v, np.float32(np.inf), dtype=np.float32)
    return float(v)

def gauss5():
    # f32 replica of reference._gaussian_kernel1d(5, 1.0)
    x = (np.arange(5, dtype=np.float32) - 2).astype(np.float32)
    g = np.exp((-x * x / np.float32(2.0)).astype(np.float32)).astype(np.float32)
    return (g / g.sum(dtype=np.float32)).astype(np.float32)

def _op_conv(n_out, n_in, taps, center, idx_map):
    """Row t of output = sum_d taps[d] * in[idx_map(t + d - center)], f64 build."""
    C = np.zeros((n_out, n_in), np.float64)
    for t in range(n_out):
        for d, w in enumerate(taps):
            s = idx_map(t + d - center)
            C[t, s] += w
    return C

def reflect_idx(i, n):
    # jnp.pad 'reflect': -1 -> 1, -2 -> 2; n -> n-2, n+1 -> n-3
    if i < 0:
        return -i
    if i >= n:
        return 2 * n - 2 - i
    return i

def clamp_idx(i, n):
    return min(max(i, 0), n - 1)

def build_vertical_ops(top: bool):
    """Return (Cvx, Cvy): [NM, XR] f32 composed vertical operators for this core."""
    g = gauss5().astype(np.float64)
    # Stage 1: gauss vertical with reflect at IMAGE edges, over x window rows.
    # blurred-v needed rows: image rows of M-1 .. M+1 -> local B = -1..529
    # local->img: top: img = local_x;  bottom: img = 492 + local_x
    # blurV local grid b = -1..529 maps to img rows (top: b, bottom: 492+b... wait
    #   bottom mag M -> img 495+M; blur rows needed img 494..1024)
    # Build on local-x axis directly with the correct edge behavior:
    #   top: local 0 == img 0 (reflect boundary at local 0); far end interior.
    #   bottom: local 531 == img 1023 (reflect boundary there); near end interior.
    NB = 531  # blur rows b = -1..529 stored t = b+1
    def xmap_top(i):   # reflect at 0 only (other end never reached out of range)
        return reflect_idx(i, 10**9) if i >= 0 else -i
    def xmap_bot(i):
        if i >= XR:
            return 2 * XR - 2 - i
        return i
    xmap = xmap_top if top else xmap_bot
    # blur b (local-x coordinate of output): top: b = t-1; bottom: b = t-1+2
    #   top: blurV[b] centered at x local row b;  b from -1..529
    #   bottom: mag M -> img 495+M -> local x = 495+M-492 = 3+M; blur rows local x = 2+M-1.. 
    #     blur grid b(local x) = 2 .. 532 for t=0..530
    off = 0 if top else 3
    Cb = np.zeros((NB, XR), np.float64)
    for t in range(NB):
        b = t - 1 + off   # local-x row this blurV output is centered on
        for d in range(5):
            s = b + d - 2
            s = xmap(s)
            assert 0 <= s < XR, (top, t, s)
            Cb[t, s] += g[d]
    # Stage 2: sobel vertical ops on the blurV grid with replicate at IMAGE edges.
    # mag M: taps at blur rows b = (M-1 .. M+1) in local-x => stored t = M-1+1..=M..M+2 - wait
    #   stored t of blur row b: t = b + 1 - off ... b_local_x = M + off + db where db=-1..1
    #   stored t = (M + off + db) - off + 1 - ... let me just: stored t corresponds to b_lx = t-1+off
    #   For mag M: need b_lx = (M+off) + db  => t = M + 1 + db
    # replicate at image edges: top: b_lx < 0 -> 0 i.e. t<0 -> t=0? replicate on blur IMG rows:
    #   top: blur img row = b_lx; replicate row<0 -> row 0 -> t index of b_lx=0 is t=1.
    #   bottom: blur img row = 492 + b_lx; replicate row>1023 -> b_lx>531 -> clamp to 531 (t=530)
    vsm = np.array([1.0, 2.0, 1.0])
    vdf = np.array([-1.0, 0.0, 1.0])
    Cvx = np.zeros((NM, XR), np.float64)
    Cvy = np.zeros((NM, XR), np.float64)
    for M in range(NM):
        for db, (wx, wy) in enumerate(zip(vsm, vdf)):
            t = M + db  # t = M+1+(db-1)
            if top:
                t = max(t, 1)       # replicate img row 0 (t=1)... t=0 is b_lx=-1 (img -1)
            else:
                t = min(t, NB - 2)  # replicate img row 1023 at far end (t=529)
            # also clamp other end (never used beyond range by construction)
            t = min(max(t, 0), NB - 1)
            Cvx[M] += wx * Cb[t]
            Cvy[M] += wy * Cb[t]
    return Cvx.astype(np.float32), Cvy.astype(np.float32)

def build_horizontal_ops():
    """(Chx, Chy): [W, W] composed horizontal operators (same both cores)."""
    g = gauss5().astype(np.float64)
    Cb = _op_conv(W_IMG, W_IMG, g, 2, lambda i: reflect_idx(i, W_IMG))
    Dif = _op_conv(W_IMG, W_IMG, [-1.0, 0.0, 1.0], 1, lambda i: clamp_idx(i, W_IMG))
    Sm = _op_conv(W_IMG, W_IMG, [1.0, 2.0, 1.0], 1, lambda i: clamp_idx(i, W_IMG))
    Chx = (Dif @ Cb).astype(np.float32)
    Chy = (Sm @ Cb).astype(np.float32)
    return Chx, Chy

# ---------------- numpy model of the per-core pipeline (for validation) -------------
def core_model(x_win, top):
    """x_win: [3, XR, 1024] f32. Returns (magout [NM,1024], edges [NM,1024])."""
    Cvx, Cvy = build_vertical_ops(top)
    Chx, Chy = build_horizontal_ops()
    gray = np.tensordot(GRAY_W, x_win.astype(np.float32), 1)  # [XR, W]
    gvx = (Cvx @ gray).astype(np.float32)
    gvy = (Cvy @ gray).astype(np.float32)
    gx = (gvx @ Chx.T).astype(np.float32)
    gy = (gvy @ Chy.T).astype(np.float32)
    m2 = gx * gx + gy * gy
    mag = np.sqrt(m2 + np.float32(1e-6)).astype(np.float32)
    magp = np.zeros((NM + 2, W_IMG + 2), np.float32)
    magp[1:-1, 1:-1] = mag
    ax, ay = np.abs(gx), np.abs(gy)
    maskH = (TAN225 * ax) >= ay
    maskV = (TAN225 * ay) > ax
    pmask = (gx * gy) >= 0
    c = magp[1:-1, 1:-1]
    up, dn = magp[0:-2, 1:-1], magp[2:, 1:-1]
    lf, rt = magp[1:-1, 0:-2], magp[1:-1, 2:]
    ul, ur = magp[0:-2, 0:-2], magp[0:-2, 2:]
    dl, dr = magp[2:, 0:-2], magp[2:, 2:]
    nbH = np.maximum(lf, rt); nbV = np.maximum(up, dn)
    nbD1 = np.maximum(dr, ul); nbD2 = np.maximum(dl, ur)
    nbsel = nbD2.copy()
    nbsel[pmask] = nbD1[pmask]
    nbsel[maskV] = nbV[maskV]
    nbsel[maskH] = nbH[maskH]
    ismax = c > nbsel
    magout = mag * ismax
    sm = magout > np.float32(0.2)
    wm = magout > np.float32(0.1)
    S = sm.copy(); W = wm
    for _ in range(ITERS):
        Sp = np.zeros((NM + 2, W_IMG + 2), bool)
        Sp[1:-1, 1:-1] = S
        dil = Sp[0:-2,0:-2]|Sp[0:-2,1:-1]|Sp[0:-2,2:]|Sp[1:-1,0:-2]|Sp[1:-1,1:-1]|Sp[1:-1,2:]|Sp[2:,0:-2]|Sp[2:,1:-1]|Sp[2:,2:]
        S = S | (W & dil)
    return magout, S.astype(np.float32)


import numpy as np
from collections import defaultdict
import concourse.bass as bass
import concourse.mybir as mybir
from concourse.masks import make_identity


F32, I32, U32, U8 = mybir.dt.float32, mybir.dt.int32, mybir.dt.uint32, mybir.dt.uint8
F32R = mybir.dt.float32r
BF16 = mybir.dt.bfloat16
OP = mybir.AluOpType
ACT = mybir.ActivationFunctionType
MASK_DT = U8

NT = 5
BTS = [128, 128, 128, 128, 17]
NCB = 8
RCH = [(1, 162), (162, 354), (354, 530)]     # conv r-slot chunks
NCH4 = [(1, 161), (161, 353), (353, 530)]    # NMS r-slot chunks (word-aligned)
PKW = [(0, 5), (5, 11), (11, 17)]            # pack word ranges per NMS chunk

XP = 535           # padded x rows: top = [0,0,0, img 0..531]; bottom = [img 492..1023, 0,0,0]
VBKS = [122, 122, 122, 122, 41]   # output rows per vertical block (sum 529)
VKS = [128, 128, 128, 128, 47]    # input rows per block, start = 122*k

def build_vplan(top):
    Cvx, Cvy = build_vertical_ops(top)
    w = np.float64(np.float32(GRAY_W[2]))  # 0.114 folded out of the DVE gray stage
    # pad to the unified 535-row local axis
    pads = []
    for C in (Cvx, Cvy):
        Cp = np.zeros((NM, XP), np.float64)
        if top:
            Cp[:, 3:3 + XR] = C
        else:
            Cp[:, 0:XR] = C
        pads.append(Cp)
    arr = np.zeros((128, 2 * NT, 128), np.float32)
    for ci, C in enumerate(pads):
        for k in range(NT):
            r0, BK, K = 122 * k, VBKS[k], VKS[k]
            sub = C[r0:r0 + BK, :]
            assert np.all(sub[:, :122 * k] == 0) and np.all(sub[:, 122 * k + K:] == 0), (top, ci, k)
            arr[0:K, ci * NT + k, 0:BK] = (w * sub[:, 122 * k:122 * k + K]).T
    return arr

def build_hplan():
    """wh [128, 16, 128]: slot ci*8+cb = dense diagonal block (input cols 128cb..+127).
    whx [8, 16, 128]: halo block: rows 0..2 = input cols 128cb-3..-1, rows 3..5 = 128cb+128..+130."""
    Chx, Chy = build_horizontal_ops()
    wh = np.zeros((128, 2 * NCB, 128), np.float32)
    whx = np.zeros((64, NCB, 128), np.float32)
    for ci, C in enumerate((Chx, Chy)):
        for cb in range(NCB):
            p0 = 128 * cb
            s = ci * NCB + cb
            b = 32 * ci
            wh[:, s, :] = C[p0:p0 + 128, p0:p0 + 128].T
            if cb > 0:
                whx[b:b + 3, cb, :] = C[p0:p0 + 128, p0 - 3:p0].T
            if cb < NCB - 1:
                whx[b + 3:b + 6, cb, :] = C[p0:p0 + 128, p0 + 128:p0 + 131].T
            assert np.all(C[p0:p0 + 128, :max(p0 - 3, 0)] == 0)
            assert np.all(C[p0:p0 + 128, p0 + 131:] == 0)
    return wh, whx

def pack_blocks(blocks, kinds):
    """kinds[i] in {'full','lo32','hi32'}; hi32 must land at k0=96, lo32/full at 0."""
    places = [None] * len(blocks)
    slots = []
    free_lo, free_hi = [], []
    for i, (b, kind) in enumerate(zip(blocks, kinds)):
        if kind == 'full':
            slots.append([])
            slots[-1].append((0, b))
            places[i] = (len(slots) - 1, 0)
        elif kind == 'lo32':
            if not free_lo:
                slots.append([])
                free_hi.append(len(slots) - 1)
                free_lo.append(len(slots) - 1)
            s = free_lo.pop(0)
            slots[s].append((0, b))
            places[i] = (s, 0)
        else:  # hi64 at k0=64
            if not free_hi:
                slots.append([])
                free_lo.append(len(slots) - 1)
                free_hi.append(len(slots) - 1)
            s = free_hi.pop(0)
            slots[s].append((64, b))
            places[i] = (s, 64)
    arr = np.zeros((128, len(slots), 128), np.float32)
    for slot, entries in enumerate(slots):
        for k0, b in entries:
            K, M = b.shape
            arr[k0:k0 + K, slot, 0:M] = b
    return arr, places

def make_core_inputs(top):
    wv = build_vplan(top)
    wh, whx = build_hplan()
    pat = np.tile(np.uint32(1) << np.arange(32, dtype=np.uint32), (128, 1))
    meta = dict(nv=wv.shape[1], nh=wh.shape[1])
    return np.ascontiguousarray(pat), wv, wh, whx, meta

def _ap(base_ap, offset_elems, dims):
    return bass.AP(base_ap.tensor, base_ap.offset + offset_elems, dims)

def r32(ap):
    return ap.bitcast(mybir.dt.float32r)

def stt_u32(nc, out, in0, scalar, in1, op0, op1):
    """scalar_tensor_tensor with an integer (u32) immediate, for bitvec ops."""
    eng = nc.vector
    return eng.add_instruction(
        mybir.InstTensorScalarPtr(
            name=nc.get_next_instruction_name(),
            is_scalar_tensor_tensor=True,
            op0=op0,
            op1=op1,
            ins=[eng.lower_ap(in0),
                 mybir.ImmediateValue(dtype=mybir.dt.uint32, value=scalar),
                 eng.lower_ap(in1)],
            outs=[eng.lower_ap(out)],
        ))

def canny_core(ctx, tc, outs, ins, meta):
    import os
    STAGE = int(os.environ.get('CANNY_STAGE', '9'))
    from contextlib import ExitStack
    nc = tc.nc
    mag_out, edges_out = outs
    x_in, wv_in, wh_in, whx_in, pat_in, ident_in = ins
    NVS, NHS = meta['nv'], meta['nh']

    consts = ctx.enter_context(tc.tile_pool(name="consts", bufs=1))
    pat_s = consts.tile([128, 32], U32)
    nc.scalar.dma_start(pat_s[:], pat_in)
    ident = consts.tile([128, 128], F32R)
    nc.scalar.dma_start(ident[:], ident_in)
    identb = consts.tile([128, 128], BF16)
    make_identity(nc, identb)

    persist = ctx.enter_context(tc.tile_pool(name="persist", bufs=1))
    magb = persist.tile([128, NCB, RD], F32)          # holds m2 = gx^2+gy^2
    magob = persist.tile([128, NCB, RD], BF16)        # final masked magnitude (bf16 ok: post-decision values)
    nc.gpsimd.memset(magb[:, :, 0:1], 0.0)
    nc.gpsimd.memset(magb[:, :, 530:532], 0.0)
    m2L_s = persist.tile([128, NCB, 194], F32, name="m2L")
    m2R_s = persist.tile([128, NCB, 194], F32, name="m2R")
    m2L = [m2L_s, m2L_s, m2L_s]
    m2R = [m2R_s, m2R_s, m2R_s]
    nc.gpsimd.memset(m2L_s[0:1, 0:1, :], 0.0)
    nc.gpsimd.memset(m2R_s[96:128, 7:8, :], 0.0)

    swp = ctx.enter_context(tc.tile_pool(name="swp", bufs=1))
    smb = swp.tile([128, NCB, PACK_ROWS + 2], U8)
    wmb = swp.tile([128, NCB, PACK_ROWS + 2], U8)
    nc.gpsimd.memset(smb[:, :, 530:546], 0)
    nc.gpsimd.memset(wmb[:, :, 530:546], 0)
    fl = ctx.enter_context(tc.tile_pool(name="fl", bufs=1))
    Sw = fl.tile([128, NCB, WSLOT], U32, tag="Sw")
    Ww = fl.tile([128, NCB, WSLOT], U32, tag="Ww")
    HL = fl.tile([128, NCB, WSLOT], U32, tag="HL")
    HR = fl.tile([128, NCB, WSLOT], U32, tag="HR")
    Hd = fl.tile([128, NCB, WSLOT], U32, tag="Hd")
    Vd = fl.tile([128, NCB, WSLOT], U32, tag="Vd")
    ta = fl.tile([128, NCB, WSLOT], U32, tag="ta")
    for t in (Sw, Ww, HL, HR, Hd, Vd, ta):
        nc.gpsimd.memset(t[:], 0)
    def flood_iter(dw):
        a, b = dw.start, dw.stop
        dm, dp = slice(a - 1, b - 1), slice(a + 1, b + 1)
        nc.sync.dma_start(out=HL[1:128, :, dw], in_=Sw[0:127, :, dw])
        nc.scalar.dma_start(out=HL[0:1, 1:8, dw], in_=Sw[127:128, 0:7, dw])
        nc.gpsimd.dma_start(out=HR[0:127, :, dw], in_=Sw[1:128, :, dw])
        nc.scalar.dma_start(out=HR[127:128, 0:7, dw], in_=Sw[0:1, 1:8, dw])
        nc.vector.tensor_tensor(out=Hd[:, :, dw], in0=Sw[:, :, dw], in1=HL[:, :, dw], op=OP.bitwise_or)
        nc.vector.tensor_tensor(out=Hd[:, :, dw], in0=Hd[:, :, dw], in1=HR[:, :, dw], op=OP.bitwise_or)
        stt_u32(nc, Vd[:, :, dw], Hd[:, :, dw], 1,
                Hd[:, :, dw], OP.logical_shift_left, OP.bitwise_or)
        stt_u32(nc, Vd[:, :, dw], Hd[:, :, dm], 31,
                Vd[:, :, dw], OP.logical_shift_right, OP.bitwise_or)
        stt_u32(nc, Vd[:, :, dw], Hd[:, :, dw], 1,
                Vd[:, :, dw], OP.logical_shift_right, OP.bitwise_or)
        stt_u32(nc, Vd[:, :, dw], Hd[:, :, dp], 31,
                Vd[:, :, dw], OP.logical_shift_left, OP.bitwise_or)
        nc.vector.tensor_tensor(out=ta[:, :, dw], in0=Ww[:, :, dw], in1=Vd[:, :, dw], op=OP.bitwise_and)
        nc.vector.tensor_tensor(out=Sw[:, :, dw], in0=Sw[:, :, dw], in1=ta[:, :, dw], op=OP.bitwise_or)

    def make_pack(pool):
        pk_l1 = pool.tile([128, NCB, 272], BF16, name="l1", tag="l1")
        pk_l2 = pool.tile([128, NCB, 136], BF16, name="l2", tag="l2")
        pk_l3 = pool.tile([128, NCB, 68], BF16, name="l3", tag="l3")
        pk_li = pool.tile([128, NCB, 34], U32, name="li", tag="li")
        pk_lsh = pool.tile([128, NCB, 17], U32, name="lsh", tag="lsh")
        return pk_l1, pk_l2, pk_l3, pk_li, pk_lsh

    def pack_part(pk, eng, srcf, dstw, w0, w1):
        l1, l2, l3, li, lsh = pk
        s_hi = srcf[:, :, 2 + 32 * w0:2 + 32 * w1:2]
        s_lo = srcf[:, :, 1 + 32 * w0:1 + 32 * w1:2]
        l1w = l1[:, :, 16 * w0:16 * w1]
        l1r = (l1[:, :, 16 * w0 + 1:16 * w1:2], l1[:, :, 16 * w0:16 * w1 - 1:2])
        l2w = l2[:, :, 8 * w0:8 * w1]
        l2r = (l2[:, :, 8 * w0 + 1:8 * w1:2], l2[:, :, 8 * w0:8 * w1 - 1:2])
        l3w = l3[:, :, 4 * w0:4 * w1]
        l3r = (l3[:, :, 4 * w0 + 1:4 * w1:2], l3[:, :, 4 * w0:4 * w1 - 1:2])
        liw = li[:, :, 2 * w0:2 * w1]
        lshw = lsh[:, :, w0:w1]
        lshr = (li[:, :, 2 * w0 + 1:2 * w1:2], li[:, :, 2 * w0:2 * w1 - 1:2])
        dw = dstw[:, :, 1 + w0:1 + w1]
        eng.scalar_tensor_tensor(out=l1w, in0=s_hi, scalar=2.0, in1=s_lo, op0=OP.mult, op1=OP.add)
        eng.scalar_tensor_tensor(out=l2w, in0=l1r[0], scalar=4.0, in1=l1r[1], op0=OP.mult, op1=OP.add)
        eng.scalar_tensor_tensor(out=l3w, in0=l2r[0], scalar=16.0, in1=l2r[1], op0=OP.mult, op1=OP.add)
        eng.scalar_tensor_tensor(out=liw, in0=l3r[0], scalar=256.0, in1=l3r[1], op0=OP.mult, op1=OP.add)
        nc.vector.tensor_scalar(out=lshw, in0=lshr[0], scalar1=16, scalar2=None, op0=OP.logical_shift_left)
        nc.vector.tensor_tensor(out=dw, in0=lshr[1], in1=lshw, op=OP.bitwise_or)
    es_mask = ctx.enter_context(ExitStack())
    maskp = es_mask.enter_context(tc.tile_pool(name="maskp", bufs=1))
    maskH = maskp.tile([128, NCB, RD], MASK_DT, name="maskH")
    maskV = maskp.tile([128, NCB, RD], MASK_DT, name="maskV")
    pmask = maskp.tile([128, NCB, RD], MASK_DT, name="pmask")

    with tc.tile_pool(name="gvt", bufs=1) as gvtp:
        gvT = [gvtp.tile([128, NCB, RD], F32, name=f"gvT{i}", tag=f"gvT{i}") for i in range(2)]
        wh_s = gvtp.tile([128, NHS, 128], F32)
        whx_s = gvtp.tile([64, NCB, 128], F32)
        gvXs = gvtp.tile([64, NCB, 194], F32, name="gvXs")
        nc.gpsimd.memset(gvXs[:], 0.0)
        # ---- vertical convs in two passes, H chunk 0 interleaved between ----
        es_x = ExitStack()
        xp = es_x.enter_context(tc.tile_pool(name="xp", bufs=1))
        gr = es_x.enter_context(tc.tile_pool(name="gr", bufs=2))
        wv_s = xp.tile([128, NVS, 128], F32)
        xtiles = {}

        def load_chunk(S):
            nrows = VKS[S]
            g = xp.tile([128, 1024], F32, name=f"gray{S}", tag=f"gray{S % 2}")
            for h in range(2):
                cs = slice(512 * h, 512 * h + 512)
                t = xp.tile([128, 3, 512], F32, name=f"xs{S}{h}", tag=f"xs{h}")
                nc.sync.dma_start(
                    out=t[0:nrows, :, :],
                    in_=_ap(x_in, 122 * S * 1024 + 512 * h,
                            [[1024, nrows], [XP * 1024, 3], [1, 512]]))
                nc.vector.scalar_tensor_tensor(out=g[0:nrows, cs], in0=t[0:nrows, 0, :],
                                               scalar=float(np.float32(0.299) / np.float32(0.587)),
                                               in1=t[0:nrows, 1, :], op0=OP.mult, op1=OP.add)
                nc.vector.scalar_tensor_tensor(out=g[0:nrows, cs], in0=g[0:nrows, cs],
                                               scalar=float(np.float32(0.587) / np.float32(0.114)),
                                               in1=t[0:nrows, 2, :], op0=OP.mult, op1=OP.add)
            xtiles[S] = g

        def vconv_pass(Ts, pvp, ptp):
            for T in Ts:
                if T not in xtiles:
                    load_chunk(T)
                BT, K = VBKS[T], VKS[T]
                for ci in range(2):
                    ps = pvp.tile([128, 1024], F32, name=f"pv{T}{ci}", tag="pv")
                    for nh in range(2):
                        cols = slice(512 * nh, 512 * nh + 512)
                        nc.tensor.matmul(ps[0:BT, cols], wv_s[0:K, ci * NT + T, 0:BT],
                                         xtiles[T][0:K, cols], start=True, stop=True)
                    grm = gr.tile([128, 1024], F32, name=f"grm{T}{ci}", tag="grm")
                    if ci == 0:
                        nc.scalar.copy(grm[0:BT, :], ps[0:BT, :])
                    else:
                        nc.vector.tensor_copy(grm[0:BT, :], ps[0:BT, :])
                    for g in range(2):
                        ptile = ptp.tile([128, 512], F32, name=f"pt{ci}{T}{g}", tag="pt")
                        for k in range(4):
                            cb = 4 * g + k
                            nc.tensor.transpose(
                                ptile[0:128, 128 * k:128 * k + BT],
                                grm[0:BT, 128 * cb:128 * cb + 128],
                                ident[0:BT, 0:BT].bitcast(F32))
                        dst_ap = gvT[ci][:, 4 * g:4 * g + 4, 1 + 122 * T:1 + 122 * T + BT]
                        src_ap = ptile[:].rearrange("p (c b) -> p c b", c=4)[:, :, 0:BT]
                        if ci == 0:
                            nc.scalar.copy(dst_ap, src_ap)
                        else:
                            nc.vector.tensor_copy(dst_ap, src_ap)

        with tc.tile_pool(name="pva", bufs=2, space="PSUM") as pva, \
             tc.tile_pool(name="pta", bufs=3, space="PSUM") as pta:
            load_chunk(0)
            nc.sync.dma_start(wv_s[:], wv_in)
            load_chunk(1)
            nc.sync.dma_start(wh_s[:], wh_in)
            nc.sync.dma_start(whx_s[:], whx_in)
            vconv_pass([0, 1, 2], pva, pta)
        if STAGE < 2:
            return
        # ---- scope 3: horizontal convs + masks + m2, per-chunk NMS overlapped ----
        NCH = NCH4
        t2c = float(np.float32(TAN225) * np.float32(TAN225))
        SM2, WM2 = thresh2(0.2), thresh2(0.1)
        with tc.tile_pool(name="nmsa", bufs=1) as na, \
             tc.tile_pool(name="nmsb", bufs=1) as nb, \
             tc.tile_pool(name="ph", bufs=2, space="PSUM") as ph:

            sqtiles = {}

            def halo_fill(ic):
                lo, hi = RCH[ic]
                CN = hi - lo
                for ci in range(2):
                    b = 32 * ci
                    # rows b+0:3 <- input cols 128cb-3..-1; rows b+3:6 <- cols 128cb+128..+130
                    nc.sync.dma_start(out=gvXs[b:b + 3, 1:8, 0:CN],
                                      in_=gvT[ci][125:128, 0:7, lo:hi])
                    nc.scalar.dma_start(out=gvXs[b + 3:b + 6, 0:7, 0:CN],
                                        in_=gvT[ci][0:3, 1:8, lo:hi])

            def conv_mm(ic, ph2=None):
                lo, hi = RCH[ic]
                CN = hi - lo
                for cb in range(NCB):
                    pool = ph if (ph2 is None or cb % 2 == 0) else ph2
                    pg = [pool.tile([128, CN], F32, name=f"pg{i}", tag=f"pg{i}") for i in range(2)]
                    for ci in range(2):
                        s = ci * NCB + cb
                        nc.tensor.matmul(pg[ci][:, 0:CN], wh_s[0:128, s, 0:128],
                                         gvT[ci][0:128, cb, lo:hi], start=True, stop=False)
                        b = 32 * ci
                        nc.tensor.matmul(pg[ci][:, 0:CN], whx_s[b:b + 6, cb, 0:128],
                                         gvXs[b:b + 6, cb, 0:CN], start=False, stop=True)
                    sqx = na.tile([128, CN], F32, name=f"sqx{ic}{cb}", tag=f"sqx{cb}")
                    sqy = na.tile([128, CN], F32, name=f"sqy{ic}{cb}", tag=f"sqy{cb}")
                    gyc = na.tile([128, CN], F32, name=f"gyc{ic}{cb}", tag=f"gyc{cb}")
                    nc.scalar.activation(sqx[:], pg[0][:, 0:CN], ACT.Square)
                    nc.scalar.activation(sqy[:], pg[1][:, 0:CN], ACT.Square)
                    nc.scalar.copy(gyc[:], pg[1][:, 0:CN])
                    nc.vector.tensor_tensor(out=gyc[:].bitcast(U32), in0=pg[0][:, 0:CN].bitcast(U32),
                                            in1=gyc[:].bitcast(U32), op=OP.bitwise_xor)
                    nc.gpsimd.tensor_tensor(out=magb[:, cb, lo:hi], in0=sqx[:], in1=sqy[:], op=OP.add)
                    sqtiles[(ic, cb)] = (sqx, sqy, gyc[:].bitcast(U32))

            def conv_post(ic):
                lo, hi = RCH[ic]
                for cb in range(NCB):
                    sqx, sqy, xr = sqtiles[(ic, cb)]
                    nc.vector.tensor_scalar(out=pmask[:, cb, lo:hi], in0=xr,
                                            scalar1=2147483648, scalar2=None, op0=OP.is_lt)
                    nc.vector.scalar_tensor_tensor(out=maskH[:, cb, lo:hi], in0=sqx[:],
                                                   scalar=t2c, in1=sqy[:],
                                                   op0=OP.mult, op1=OP.is_ge)
                    nc.vector.scalar_tensor_tensor(out=maskV[:, cb, lo:hi], in0=sqy[:],
                                                   scalar=t2c, in1=sqx[:],
                                                   op0=OP.mult, op1=OP.is_gt)

            def shift_dmas(jc):
                nlo, nhi = NCH[jc]
                ra, rb = nlo - 1, nhi + 1
                CNH = rb - ra
                nc.sync.dma_start(out=m2L[jc][1:128, :, 0:CNH], in_=magb[0:127, :, ra:rb])
                nc.sync.dma_start(out=m2L[jc][0:1, 1:8, 0:CNH], in_=magb[127:128, 0:7, ra:rb])
                nc.scalar.dma_start(out=m2R[jc][0:127, :, 0:CNH], in_=magb[1:128, :, ra:rb])
                nc.scalar.dma_start(out=m2R[jc][127:128, 0:7, 0:CNH], in_=magb[0:1, 1:8, ra:rb])

            def nms_chunk(jc):
                nlo, nhi = NCH[jc]
                CN = nhi - nlo
                ra = nlo - 1
                c0 = slice(nlo, nhi)
                cm = slice(nlo - 1, nhi - 1)
                cp_ = slice(nlo + 1, nhi + 1)
                lc0 = slice(1, 1 + CN)
                lcm = slice(0, CN)
                lcp = slice(2, 2 + CN)
                L, R = m2L[jc], m2R[jc]
                nbsel_t = nb.tile([128, NCB, CN], F32, name=f"nbsel{jc}", tag="nbsel")
                tmp_t = nb.tile([128, NCB, CN], F32, name=f"tmp{jc}", tag="tmp")
                nbsel, tmp = nbsel_t[:], tmp_t[:]
                msk = nbsel
                nc.vector.tensor_tensor(out=nbsel, in0=L[:, :, lcp], in1=R[:, :, lcm], op=OP.max)
                nc.vector.tensor_tensor(out=tmp, in0=R[:, :, lcp], in1=L[:, :, lcm], op=OP.max)
                nc.vector.copy_predicated(nbsel, pmask[:, :, c0], tmp)
                nc.vector.tensor_tensor(out=tmp, in0=magb[:, :, cm], in1=magb[:, :, cp_], op=OP.max)
                nc.vector.copy_predicated(nbsel, maskV[:, :, c0], tmp)
                nc.vector.tensor_tensor(out=tmp, in0=L[:, :, lc0], in1=R[:, :, lc0], op=OP.max)
                nc.vector.copy_predicated(nbsel, maskH[:, :, c0], tmp)
                nc.vector.tensor_tensor(out=tmp, in0=magb[:, :, c0], in1=nbsel, op=OP.is_gt)
                nc.vector.scalar_tensor_tensor(out=msk, in0=magb[:, :, c0], scalar=1e-6,
                                               in1=tmp, op0=OP.add, op1=OP.mult)
                nc.vector.tensor_scalar(out=smb[:, :, c0], in0=msk,
                                        scalar1=SM2, scalar2=None, op0=OP.is_gt)
                nc.vector.tensor_scalar(out=wmb[:, :, c0], in0=msk,
                                        scalar1=WM2, scalar2=None, op0=OP.is_gt)
                nc.scalar.activation(magob[:, :, c0], msk, ACT.Sqrt)

            pk = make_pack(nb)
            halo_fill(0)
            conv_mm(0)
            with tc.tile_pool(name="pvb", bufs=1, space="PSUM") as pvb, \
                 tc.tile_pool(name="ptb", bufs=2, space="PSUM") as ptb:
                vconv_pass([3, 4], pvb, ptb)
            conv_post(0)
            shift_dmas(0)
            nms_chunk(0)
            pack_part(pk, nc.vector, smb, Sw, *PKW[0])
            pack_part(pk, nc.vector, wmb, Ww, *PKW[0])
            with tc.tile_pool(name="ph2", bufs=2, space="PSUM") as ph2:
                for jc in (1, 2):
                    halo_fill(jc)
                    conv_mm(jc, ph2)
                    conv_post(jc)
                    shift_dmas(jc)
                    nms_chunk(jc)
                    if jc == 2:
                        flood_iter(slice(1, 11))   # early iter 0 on packed words
                    pack_part(pk, nc.vector, smb, Sw, *PKW[jc])
                    pack_part(pk, nc.vector, wmb, Ww, *PKW[jc])
        es_x.close()
    es_mask.close()
    if STAGE < 4:
        return
    # ---- scope 4: flood fill, unpack, transpose-out, DMA out ----
    if True:
        es_out = ExitStack()
        stp = es_out.enter_context(tc.tile_pool(name="st", bufs=1))
        pto = es_out.enter_context(tc.tile_pool(name="pto", bufs=4, space="PSUM"))
        def emit_output(oi, src_t, dst, r0off, bf=False):
            stage = stp.tile([128, NT, 1024], F32, name=f"stage{oi}", tag="stage")
            for T in range(NT):
                BT = BTS[T]
                for g in range(2):
                    ptile = pto.tile([128, 512], BF16 if bf else F32, name=f"pto{oi}{T}{g}", tag="pto")
                    for k in range(4):
                        cb = 4 * g + k
                        if bf:
                            nc.tensor.transpose(
                                ptile[0:BT, 128 * k:128 * k + 128],
                                src_t[:, cb, r0off + 128 * T: r0off + 128 * T + BT],
                                identb[:])
                        else:
                            nc.tensor.transpose(
                                r32(ptile[0:BT, 128 * k:128 * k + 128]),
                                r32(src_t[:, cb, r0off + 128 * T: r0off + 128 * T + BT]),
                                ident[:])
                    nc.scalar.copy(stage[0:BT, T, 512 * g:512 * g + 512], ptile[0:BT, :])
            for T in range(4):
                nc.sync.dma_start(
                    out=_ap(dst, 128 * T * 1024, [[1024, 128], [1, 1024]]),
                    in_=stage[:, T, :])
            nc.sync.dma_start(
                out=_ap(dst, 512 * 1024, [[1024, 17], [1, 1024]]),
                in_=stage[0:17, 4, :])
        emit_output(0, magob, mag_out, 1, bf=True)
        for it in range(1, ITERS):
            flood_iter(slice(1, 18))

        if STAGE < 6:
            return
        edgesT = stp.tile([128, NCB, PACK_ROWS], F32R, tag="edgesT")
        eti = stp.tile([128, NCB, PACK_ROWS], U32, tag="eti")
        stage = stp.tile([128, NT, 1024], F32, name="stageE", tag="stage")
        pap = pat_s[:, :]
        for T in (4, 0, 1, 2, 3):
            BT = BTS[T]
            W = 4 if T < 4 else 1
            sap = Sw[:, :, 1 + 4 * T:1 + 4 * T + W]
            bits_in = bass.AP(sap.tensor, sap.offset, list(sap.ap) + [[0, 32]])
            pat_bc = bass.AP(pap.tensor, pap.offset, [list(pap.ap[0]), [0, NCB], [0, W], list(pap.ap[1])])
            rsl = slice(128 * T, 128 * T + 32 * W)
            nc.vector.tensor_tensor(out=eti[:, :, rsl].rearrange("p c (j k) -> p c j k", k=32),
                                    in0=bits_in, in1=pat_bc, op=OP.bitwise_and)
            nc.vector.tensor_scalar(out=edgesT[:, :, rsl], in0=eti[:, :, rsl],
                                    scalar1=0, scalar2=None, op0=OP.not_equal)
            for g in range(2):
                ptile = pto.tile([128, 512], F32, name=f"ptoE{T}{g}", tag="pto")
                for k in range(4):
                    cb = 4 * g + k
                    nc.tensor.transpose(
                        r32(ptile[0:BT, 128 * k:128 * k + 128]),
                        r32(edgesT[:, cb, 128 * T:128 * T + BT]),
                        ident[:])
                nc.scalar.copy(stage[0:BT, T, 512 * g:512 * g + 512], ptile[0:BT, :])
            nc.sync.dma_start(
                out=_ap(edges_out, 128 * T * 1024, [[1024, BT], [1, 1024]]),
                in_=stage[0:BT, T, :])
        es_out.close()


_CACHE = {}

def _build():
    if 'nc' in _CACHE:
        return
    import concourse.bacc as bacc
    import concourse.mybir as mybir
    import concourse.tile as tile
    from contextlib import ExitStack
    patT, wvT, whT, whxT, metaT = make_core_inputs(True)
    patB, wvB, whB, whxB, metaB = make_core_inputs(False)
    assert metaT == metaB
    nc = bacc.Bacc("TRN2", target_bir_lowering=False, debug=False)
    x = nc.dram_tensor("x", [3, XP, 1024], mybir.dt.float32, kind="ExternalInput")
    wv = nc.dram_tensor("wv", list(wvT.shape), mybir.dt.float32, kind="ExternalInput")
    wh = nc.dram_tensor("wh", list(whT.shape), mybir.dt.float32, kind="ExternalInput")
    whx = nc.dram_tensor("whx", list(whxT.shape), mybir.dt.float32, kind="ExternalInput")
    pat = nc.dram_tensor("pat", [128, 32], mybir.dt.uint32, kind="ExternalInput")
    identt = nc.dram_tensor("ident", [128, 128], mybir.dt.float32r, kind="ExternalInput")
    mag_o = nc.dram_tensor("mag_o", [529, 1024], mybir.dt.float32, kind="ExternalOutput")
    edges_o = nc.dram_tensor("edges_o", [529, 1024], mybir.dt.float32, kind="ExternalOutput")
    with ExitStack() as ctx:
        tc = ctx.enter_context(tile.TileContext(nc))
        canny_core(ctx, tc, [mag_o.ap(), edges_o.ap()],
                   [x.ap(), wv.ap(), wh.ap(), whx.ap(), pat.ap(), identt.ap()], metaT)
    nc.finalize()
    _CACHE.update(nc=nc, weights=dict(top=(patT, wvT, whT, whxT), bot=(patB, wvB, whB, whxB)))

def kernel(x):
    _build()
    from concourse.bass_utils import run_bass_kernel_spmd
    nc = _CACHE['nc']
    x = np.ascontiguousarray(np.asarray(x, dtype=np.float32))
    B = x.shape[0]
    in_maps = []
    for core in range(8):
        b, half = core // 2, core % 2
        top = (half == 0)
        pad = np.zeros((3, 3, 1024), np.float32)
        if top:
            xw = np.concatenate([pad, x[b, :, 0:532, :]], axis=1)
        else:
            xw = np.concatenate([x[b, :, 492:1024, :], pad], axis=1)
        patc, wvc, whc, whxc = _CACHE['weights']['top' if top else 'bot']
        in_maps.append({"x": np.ascontiguousarray(xw), "wv": wvc, "wh": whc, "whx": whxc,
                        "pat": patc, "ident": np.eye(128, dtype=np.float32)})
    res = run_bass_kernel_spmd(nc, in_maps, core_ids=list(range(8)))
    mag = np.zeros((B, 1, 1024, 1024), np.float32)
    edges = np.zeros((B, 1, 1024, 1024), np.float32)
    for core in range(8):
        b, half = core // 2, core % 2
        r = res.results[core]
        if half == 0:
            mag[b, 0, 0:512] = r["mag_o"][0:512]
            edges[b, 0, 0:512] = r["edges_o"][0:512]
        else:
            mag[b, 0, 512:1024] = r["mag_o"][17:529]
            edges[b, 0, 512:1024] = r["edges_o"][17:529]
    return mag, edges

